# revision 20
# baseline (speedup 1.0000x reference)
"""Trainium2 Bass kernel for nn_FC_mono_12086037971055 (dense_mlp).

Computation (per batch b of x: (T=4096, C=256)):
  norm_x = x / sqrt(sum_t x^2 + 1e-7)          (column-normalize over T)
  cor    = norm_x^T @ norm_x                   (C x C Gram of correlations)
  att    = MLP(cor) elementwise                (1->4 elu ->4 elu -> BN -> 1 sigmoid)
  att    = att / (sum_axis1(att) + 1e-7)       (column-normalize)
  out    = x + x @ (offdiag * att)
plus a scalar monotonicity penalty from 21 keypoints (computed host-side: it
depends only on the tiny MLP params).

Sharding: data-parallel over batch B=32 -> 4 batches on each of 8 cores.
All MLP/BN parameters are folded on the host into scalar immediates that are
baked into the Bass program (BN is affine at inference, so it folds into the
final dense layer).

Kernel strategy per batch (one NeuronCore):
  - G = X^T X with raw X via PE matmul in float32r (full rate at N=256).
    Column norms Q_c are G's diagonal, extracted with an eye-mask reduce.
  - cor = s_c * s_d * G with s = 1/sqrt(Q+eps).  The per-free-dim scale is
    applied via PE transposes of row-scaled blocks (G is symmetric, so only
    blocks 00, 10, 11 are needed; 10 comes from transposing row-scaled 01).
  - MLP runs elementwise on one packed (128, 384) tile.
    elu(u) = min(exp(u) - 1, relu(u)), exp/relu/sigmoid on ScalarE.
  - att column-normalize: row-sums (symmetry) -> R = 1/(S+eps); the
    per-column application is again row-scale + PE transpose.  The offdiag
    mask and the +Identity fold (out = X @ (offdiag*att + I)) are applied
    during the PSUM->SBUF copies.
  - out rows: lhsT = X^T (built with PE transposes of X), rhs = A''.
"""

import numpy as np

import concourse.bass as bass
import concourse.tile as tile
from concourse import bacc, bass_utils, mybir

F32 = mybir.dt.float32
F32R = mybir.dt.float32r
F16 = mybir.dt.float16
AF = mybir.ActivationFunctionType
ALU = mybir.AluOpType
X_AX = mybir.AxisListType.X

N_CORES = 8
B_TOTAL = 32
B_PER_CORE = B_TOTAL // N_CORES   # 4
T = 4096
C = 256
P = 128
NT = T // P                        # 32 t-blocks
NCHUNK = 4                         # x streamed in 4 chunks per batch
LT = NT // NCHUNK                  # 8 t-blocks per chunk
BN_EPS = 1e-3
EPS = 1e-7
PENALTY_RATE = 10.0


def _fold_params(inputs):
    """Fold BN into the projection layer; return plain python floats."""
    W1 = np.asarray(inputs["W1"], np.float64)     # (1, 4)
    b1 = np.asarray(inputs["b1"], np.float64)     # (4,)
    W2 = np.asarray(inputs["W2"], np.float64)     # (4, 4)
    b2 = np.asarray(inputs["b2"], np.float64)     # (4,)
    gamma = np.asarray(inputs["gamma"], np.float64)
    beta = np.asarray(inputs["beta"], np.float64)
    mmean = np.asarray(inputs["mmean"], np.float64)
    mvar = np.asarray(inputs["mvar"], np.float64)
    Wp = np.asarray(inputs["Wp"], np.float64)     # (4, 1)
    bp = np.asarray(inputs["bp"], np.float64)     # (1,)

    a = gamma / np.sqrt(mvar + BN_EPS)
    wp2 = Wp[:, 0] * a
    bp2 = bp[0] + np.sum(Wp[:, 0] * (beta - mmean * a))
    return (
        [float(v) for v in W1[0]],
        [float(v) for v in b1],
        [[float(W2[i, j]) for j in range(4)] for i in range(4)],
        [float(v) for v in b2],
        [float(v) for v in wp2],
        float(bp2),
    )


class _St:
    pass


STAGE_RANGES = []


def _mark(nc, label):
    STAGE_RANGES.append((label, int(nc.next_id())))


def _build_kernel(tc, o_d, x_d, w1, b1, W2, b2, wp2, bp2):
    nc = tc.nc

    import contextlib
    ctx = contextlib.ExitStack()
    consts = ctx.enter_context(tc.tile_pool(name="consts", bufs=1))
    xin = ctx.enter_context(tc.tile_pool(name="xin", bufs=3))
    xrp = ctx.enter_context(tc.tile_pool(name="xrp", bufs=3))
    xtp = ctx.enter_context(tc.tile_pool(name="xtp", bufs=6))
    mlp = ctx.enter_context(tc.tile_pool(name="mlp", bufs=2))
    small = ctx.enter_context(tc.tile_pool(name="small", bufs=2))
    apool = ctx.enter_context(tc.tile_pool(name="apool", bufs=2))
    ostage = ctx.enter_context(tc.tile_pool(name="ostage", bufs=2))
    pp_gram = ctx.enter_context(tc.tile_pool(name="pp_gram", bufs=1, space="PSUM"))
    pp_xt = ctx.enter_context(tc.tile_pool(name="pp_xt", bufs=2, space="PSUM"))
    pp_mlp = ctx.enter_context(tc.tile_pool(name="pp_mlp", bufs=2, space="PSUM"))
    pp_out = ctx.enter_context(tc.tile_pool(name="pp_out", bufs=2, space="PSUM"))

    # constants: identity (also the eye mask) and offdiag = 1 - eye
    from concourse.masks import make_identity
    ident = consts.tile([P, P], F32, tag="ident")
    make_identity(nc, ident)
    identr = consts.tile([P, P], F32R, tag="identr")
    nc.vector.tensor_copy(identr, ident)
    od = consts.tile([P, P], F32, tag="od")
    nc.vector.tensor_scalar(
        out=od, in0=ident, scalar1=-1.0, scalar2=1.0, op0=ALU.mult, op1=ALU.add
    )

    # (128,1) constant tiles used as activation biases
    bias_tiles = {}

    def cbias(val):
        v = float(val)
        if v not in bias_tiles:
            t = consts.tile(
                [P, 1], F32, tag=f"cb{len(bias_tiles)}",
                name=f"cb{len(bias_tiles)}",
            )
            nc.vector.memset(t, v)
            bias_tiles[v] = t
        return bias_tiles[v]

    def r32(ap):
        return ap.bitcast(F32R)

    # ---------------- stage A: load, Gram, X^T, cor -> m_in ----------------
    def stage_a(b, st):
        # load x[b] in 4 chunks of (1024, 256)
        st.X = []
        for g in range(NCHUNK):
            xg = xin.tile([P, LT, C], F32, tag="xin")
            src = x_d[b, g * LT * P:(g + 1) * LT * P, :].rearrange(
                "(lt p) c -> p lt c", p=P
            )
            nc.sync.dma_start(out=xg, in_=src)
            # round to f32r for full-rate PE consumption (GPSIMD is idle)
            xr = xrp.tile([P, LT, C], F32R, tag="xr")
            nc.gpsimd.tensor_copy(xr, xg)
            st.X.append(xr)

        st.XT = [
            xtp.tile([P, T], F32R, tag="xt", name=f"xt_b{b}_c{cb}")
            for cb in range(2)
        ]

        G0 = pp_gram.tile([P, C], F32, tag="g0")
        G1 = pp_gram.tile([P, C], F32, tag="g1")

        for g in range(NCHUNK):
            xg = st.X[g]
            for lt in range(LT):
                k = g * LT + lt
                xa = xg[:, lt, :]
                nc.tensor.matmul(
                    G0[:, :], xa[:, 0:P], xa,
                    start=(k == 0), stop=(k == NT - 1),
                )
                nc.tensor.matmul(
                    G1[:, :], xa[:, P:C], xa,
                    start=(k == 0), stop=(k == NT - 1),
                )
            # transposes of this chunk into X^T
            for cb in range(2):
                for h in range(2):
                    tg = pp_xt.tile([P, 4 * P], F32R, tag="xtT")
                    for i in range(4):
                        lt = h * 4 + i
                        nc.tensor.transpose(
                            tg[:, i * P:(i + 1) * P],
                            xg[:, lt, cb * P:(cb + 1) * P],
                            identr,
                        )
                    t0 = (g * LT + h * 4) * P
                    dst = st.XT[cb][:, t0:t0 + 4 * P]
                    if (g * 2 + cb + h) % 2 == 0:
                        nc.scalar.copy(dst, tg)
                    else:
                        nc.vector.tensor_copy(dst, tg)

        # copy G out of PSUM immediately so the next batch's Gram can start
        Gs0 = small.tile([P, C], F32, tag="gs0")
        Gs1 = small.tile([P, P], F32, tag="gs1")
        nc.scalar.copy(Gs0, G0)
        nc.scalar.copy(Gs1, G1[:, P:C])

        # column norms from the Gram diagonal
        scr = small.tile([P, P], F32, tag="scr")
        Q0 = small.tile([P, 1], F32, tag="q0")
        Q1 = small.tile([P, 1], F32, tag="q1")
        nc.vector.tensor_mul(scr, Gs0[:, 0:P], ident)
        nc.vector.tensor_reduce(Q0, scr, axis=X_AX, op=ALU.add)
        scr2 = small.tile([P, P], F32, tag="scr2")
        nc.vector.tensor_mul(scr2, Gs1, ident)
        nc.vector.tensor_reduce(Q1, scr2, axis=X_AX, op=ALU.add)
        sq0 = small.tile([P, 1], F32, tag="sq0")
        sq1 = small.tile([P, 1], F32, tag="sq1")
        nc.scalar.activation(sq0, Q0, AF.Ln, bias=cbias(EPS), scale=1.0)
        nc.scalar.activation(sq1, Q1, AF.Ln, bias=cbias(EPS), scale=1.0)
        s0 = small.tile([P, 1], F32, tag="s0")
        s1 = small.tile([P, 1], F32, tag="s1")
        nc.scalar.activation(s0, sq0, AF.Exp, bias=cbias(0.0), scale=-0.5)
        nc.scalar.activation(s1, sq1, AF.Exp, bias=cbias(0.0), scale=-0.5)

        # cor blocks: row-scale, transpose, scale again on the copy out
        u00 = small.tile([P, P], F32, tag="u00")
        u01 = small.tile([P, P], F32, tag="u01")
        u11 = small.tile([P, P], F32, tag="u11")
        nc.vector.tensor_scalar_mul(u00, Gs0[:, 0:P], s0)
        nc.vector.tensor_scalar_mul(u01, Gs0[:, P:C], s0)
        nc.vector.tensor_scalar_mul(u11, Gs1, s1)
        tgc = pp_mlp.tile([P, 3 * P], F32, tag="mlpT")
        nc.tensor.transpose(tgc[:, 0:P], u00, ident)
        nc.tensor.transpose(tgc[:, P:2 * P], u01, ident)
        nc.tensor.transpose(tgc[:, 2 * P:3 * P], u11, ident)
        m_in = mlp.tile([P, 3 * P], F32, tag="m_in", bufs=3)
        nc.vector.tensor_scalar_mul(m_in[:, 0:P], tgc[:, 0:P], s0)
        nc.vector.tensor_scalar_mul(m_in[:, P:2 * P], tgc[:, P:2 * P], s1)
        nc.vector.tensor_scalar_mul(m_in[:, 2 * P:3 * P], tgc[:, 2 * P:3 * P], s1)
        st.m_in = m_in

    # ---------------- stage B: MLP + normalize -> A'' ----------------
    def stage_b(b, st):
        m_in = st.m_in
        W = 3 * P

        def elu_pair(src, scale, bias):
            E = mlp.tile([P, W], F32, tag="e")
            R = mlp.tile([P, W], F32, tag="r")
            nc.scalar.activation(E, src, AF.Exp, bias=bias, scale=scale)
            nc.scalar.activation(R, src, AF.Relu, bias=bias, scale=scale)
            h = mlp.tile([P, W], F32, tag="h")
            nc.vector.scalar_tensor_tensor(
                h, in0=E, scalar=-1.0, in1=R, op0=ALU.add, op1=ALU.min
            )
            return h

        H1 = []
        for i in range(4):
            E = mlp.tile([P, W], F16, tag="e")
            R = mlp.tile([P, W], F16, tag="r")
            nc.scalar.activation(E, m_in, AF.Exp, bias=cbias(b1[i]), scale=w1[i])
            import os as _os
            if b1[i] == 0.0 and _os.environ.get("K_R1", "act") == "dve":
                nc.vector.tensor_scalar(
                    out=R, in0=m_in, scalar1=w1[i], scalar2=0.0,
                    op0=ALU.mult, op1=ALU.max,
                )
            else:
                nc.scalar.activation(R, m_in, AF.Relu, bias=cbias(b1[i]), scale=w1[i])
            h = mlp.tile([P, W], F16, tag=f"h1_{i}", bufs=1)
            nc.vector.scalar_tensor_tensor(
                h, in0=E, scalar=-1.0, in1=R, op0=ALU.add, op1=ALU.min
            )
            H1.append(h)

        H2 = []
        for j in range(4):
            import os as _os
            _l2g = _os.environ.get("K_L2G", "dve")
            if _l2g == "odd":
                eng = nc.gpsimd if j % 2 == 1 else nc.vector
            elif _l2g == "j3":
                eng = nc.gpsimd if j == 3 else nc.vector
            else:
                eng = nc.vector
            cur = mlp.tile([P, W], F16, tag="u")
            eng.tensor_scalar(
                out=cur, in0=H1[0], scalar1=W2[0][j], scalar2=b2[j],
                op0=ALU.mult, op1=ALU.add,
            )
            for i in range(1, 4):
                nxt = mlp.tile([P, W], F16, tag="u")
                eng.scalar_tensor_tensor(
                    nxt, in0=H1[i], scalar=W2[i][j], in1=cur,
                    op0=ALU.mult, op1=ALU.add,
                )
                cur = nxt
            E = mlp.tile([P, W], F16, tag="e")
            R = mlp.tile([P, W], F16, tag="r")
            nc.scalar.activation(E, cur, AF.Exp, bias=cbias(0.0))
            nc.scalar.activation(R, cur, AF.Relu, bias=cbias(0.0))
            h = mlp.tile([P, W], F16, tag=f"h2_{j}", bufs=1)
            nc.vector.scalar_tensor_tensor(
                h, in0=E, scalar=-1.0, in1=R, op0=ALU.add, op1=ALU.min
            )
            H2.append(h)

        import os as _os
        _l3 = nc.gpsimd if _os.environ.get("K_L3", "dve") == "gps" else nc.vector
        cur = mlp.tile([P, W], F16, tag="u")
        _l3.tensor_scalar(
            out=cur, in0=H2[0], scalar1=wp2[0], scalar2=bp2,
            op0=ALU.mult, op1=ALU.add,
        )
        for j in range(1, 4):
            nxt = mlp.tile([P, W], F16, tag="u")
            _l3.scalar_tensor_tensor(
                nxt, in0=H2[j], scalar=wp2[j], in1=cur, op0=ALU.mult, op1=ALU.add
            )
            cur = nxt
        eneg = mlp.tile([P, W], F16, tag="e")
        nc.scalar.activation(eneg, cur, AF.Exp, bias=cbias(0.0), scale=-1.0)
        wden = mlp.tile([P, W], F32, tag="u")
        nc.vector.tensor_scalar_add(wden, eneg, 1.0)
        a_out = mlp.tile([P, W], F32, tag="a_out")
        nc.vector.reciprocal(a_out, wden)

        # A01 = A10^T  (att pre-normalization is symmetric)
        t01p = pp_mlp.tile([P, 3 * P], F32, tag="mlpT")
        nc.tensor.transpose(t01p[:, 0:P], a_out[:, P:2 * P], ident)
        t01 = small.tile([P, P], F32, tag="t01")
        S01 = small.tile([P, 1], F32, tag="s01")
        nc.scalar.copy(t01, t01p[:, 0:P])
        nc.vector.tensor_reduce(S01, t01, axis=X_AX, op=ALU.add)

        # column sums via row sums (symmetry)
        Sa = small.tile([P, 1], F32, tag="sa")
        nc.vector.tensor_reduce(Sa, a_out[:, 0:P], axis=X_AX, op=ALU.add)
        Sc0 = small.tile([P, 1], F32, tag="sc0")
        nc.vector.tensor_add(Sc0, Sa, S01)
        Sc1 = small.tile([P, 1], F32, tag="sc1")
        nc.vector.tensor_reduce(Sc1, a_out[:, P:3 * P], axis=X_AX, op=ALU.add)
        R0 = small.tile([P, 1], F32, tag="r0")
        R1 = small.tile([P, 1], F32, tag="r1")
        t0 = small.tile([P, 1], F32, tag="t0")
        t1 = small.tile([P, 1], F32, tag="t1")
        nc.vector.tensor_scalar_add(t0, Sc0, EPS)
        nc.vector.tensor_scalar_add(t1, Sc1, EPS)
        nc.vector.reciprocal(R0, t0)
        nc.vector.reciprocal(R1, t1)

        # U = R * (A * offdiag-mask), blockwise
        n00 = small.tile([P, P], F32, tag="n00")
        n11 = small.tile([P, P], F32, tag="n11")
        nc.vector.tensor_mul(n00, a_out[:, 0:P], od)
        nc.vector.tensor_mul(n11, a_out[:, 2 * P:3 * P], od)
        v00 = small.tile([P, P], F32, tag="v00")
        v01 = small.tile([P, P], F32, tag="v01")
        v10 = small.tile([P, P], F32, tag="v10")
        v11 = small.tile([P, P], F32, tag="v11")
        nc.vector.tensor_scalar_mul(v00, n00, R0)
        nc.vector.tensor_scalar_mul(v01, t01, R0)
        nc.vector.tensor_scalar_mul(v10, a_out[:, P:2 * P], R1)
        nc.vector.tensor_scalar_mul(v11, n11, R1)

        # A'' = U^T + I
        p0 = pp_mlp.tile([P, 3 * P], F32, tag="mlpT")
        nc.tensor.transpose(p0[:, 0:P], v00, ident)
        nc.tensor.transpose(p0[:, P:2 * P], v10, ident)
        p1 = pp_mlp.tile([P, 3 * P], F32, tag="mlpT")
        nc.tensor.transpose(p1[:, 0:P], v01, ident)
        nc.tensor.transpose(p1[:, P:2 * P], v11, ident)
        A0 = apool.tile([P, C], F32R, tag="A0")
        A1 = apool.tile([P, C], F32R, tag="A1")
        nc.vector.scalar_tensor_tensor(
            A0[:, 0:P], in0=p0[:, 0:P], scalar=1.0, in1=ident,
            op0=ALU.mult, op1=ALU.add,
        )
        nc.scalar.copy(A0[:, P:C], p0[:, P:2 * P])
        nc.scalar.copy(A1[:, 0:P], p1[:, 0:P])
        nc.vector.scalar_tensor_tensor(
            A1[:, P:C], in0=p1[:, P:2 * P], scalar=1.0, in1=ident,
            op0=ALU.mult, op1=ALU.add,
        )
        st.A = (A0, A1)

    # ---------------- stage C: out = X @ A'' ----------------
    def stage_c(b, st):
        A0, A1 = st.A
        XT0, XT1 = st.XT
        for g in range(NCHUNK):
            ost = ostage.tile([P, LT, C], F32, tag="ost")
            for lt in range(LT):
                tb = g * LT + lt
                po = pp_out.tile([P, C], F32, tag="po")
                nc.tensor.matmul(
                    po, XT0[:, tb * P:(tb + 1) * P], A0,
                    start=True, stop=False,
                )
                nc.tensor.matmul(
                    po, XT1[:, tb * P:(tb + 1) * P], A1,
                    start=False, stop=True,
                )
                if lt % 2 == 0:
                    nc.scalar.copy(ost[:, lt, :], po)
                else:
                    nc.vector.tensor_copy(ost[:, lt, :], po)
            dst = o_d[b, g * LT * P:(g + 1) * LT * P, :].rearrange(
                "(lt p) c -> p lt c", p=P
            )
            nc.sync.dma_start(out=dst, in_=ost)

    # ---------------- software-pipelined emission ----------------
    import os as _os
    nb = int(_os.environ.get("KERNEL_NBATCH", str(B_PER_CORE)))
    _ = _os
    sts = [_St() for _ in range(B_PER_CORE)]
    del STAGE_RANGES[:]
    _stage_a, _stage_b, _stage_c = stage_a, stage_b, stage_c

    def stage_a(b, st):
        _mark(nc, f"A{b}")
        _stage_a(b, st)

    def stage_b(b, st):
        _mark(nc, f"B{b}")
        _stage_b(b, st)

    def stage_c(b, st):
        _mark(nc, f"C{b}")
        _stage_c(b, st)

    if nb == 1:
        stage_a(0, sts[0]); stage_b(0, sts[0]); stage_c(0, sts[0])
    elif nb == 2:
        stage_a(0, sts[0]); stage_a(1, sts[1])
        stage_b(0, sts[0]); stage_c(0, sts[0])
        stage_b(1, sts[1]); stage_c(1, sts[1])
    elif _os.environ.get("K_PIPE", "3") == "3":
        stage_a(0, sts[0])
        stage_a(1, sts[1])
        stage_b(0, sts[0])
        stage_a(2, sts[2])
        stage_c(0, sts[0])
        stage_b(1, sts[1])
        stage_a(3, sts[3])
        stage_c(1, sts[1])
        stage_b(2, sts[2])
        stage_c(2, sts[2])
        stage_b(3, sts[3])
        stage_c(3, sts[3])
    else:
        stage_a(0, sts[0])
        stage_a(1, sts[1])
        stage_b(0, sts[0])
        stage_c(0, sts[0])
        stage_a(2, sts[2])
        stage_b(1, sts[1])
        stage_c(1, sts[1])
        stage_a(3, sts[3])
        stage_b(2, sts[2])
        stage_c(2, sts[2])
        stage_b(3, sts[3])
        stage_c(3, sts[3])

    ctx.close()


_ACT_TABLES_PATCHED = False


def _pin_act_table():
    """Force the act-table chooser onto natural_log_exp_and_others (which
    covers Copy/Ln/Exp/Relu) so the whole kernel needs ONE table load
    instead of thrashing between exp_and_others and natural_log."""
    global _ACT_TABLES_PATCHED
    if _ACT_TABLES_PATCHED:
        return
    from concourse import hw_specs
    import concourse.bacc as bacc_mod
    orig = hw_specs.get_activation_tables
    mine = {AF.Copy, AF.Ln, AF.Exp, AF.Relu, AF.Identity}
    keep = "natural_log_exp_and_others"

    def patched(arch):
        tabs = orig(arch)
        if keep not in tabs or not mine <= tabs[keep]:
            return tabs
        return {
            name: (s if name == keep else s - mine)
            for name, s in tabs.items()
        }

    bacc_mod.get_activation_tables = patched
    _ACT_TABLES_PATCHED = True


def build_program(inputs):
    """Build + compile the SPMD Bass program with folded params baked in."""
    _pin_act_table()
    w1, b1, W2, b2, wp2, bp2 = _fold_params(inputs)
    nc = bacc.Bacc(
        "TRN2",
        target_bir_lowering=False,
        debug=False,
        enable_asserts=False,
        num_devices=N_CORES,
    )
    x_d = nc.dram_tensor("x", (B_PER_CORE, T, C), F32, kind="ExternalInput").ap()
    o_d = nc.dram_tensor("out", (B_PER_CORE, T, C), F32, kind="ExternalOutput").ap()
    with tile.TileContext(nc) as tc:
        _build_kernel(tc, o_d, x_d, w1, b1, W2, b2, wp2, bp2)
    nc.compile()
    return nc


def run_device(nc, x, trace=False, **kw):
    """Run the compiled program over the 8 cores; return (out, results)."""
    x = np.asarray(x, np.float32)
    in_maps = [
        {"x": np.ascontiguousarray(x[c * B_PER_CORE:(c + 1) * B_PER_CORE])}
        for c in range(N_CORES)
    ]
    res = bass_utils.run_bass_kernel_spmd(
        nc, in_maps, core_ids=list(range(N_CORES)), trace=trace, **kw
    )
    out = np.concatenate(
        [res.results[c]["out"] for c in range(N_CORES)], axis=0
    )
    return out, res


def host_penalty(inputs):
    """Monotonicity penalty on 21 keypoints; float32 math mirroring reference."""
    f32 = np.float32
    W1 = np.asarray(inputs["W1"], f32)
    b1 = np.asarray(inputs["b1"], f32)
    W2 = np.asarray(inputs["W2"], f32)
    b2 = np.asarray(inputs["b2"], f32)
    gamma = np.asarray(inputs["gamma"], f32)
    beta = np.asarray(inputs["beta"], f32)
    mmean = np.asarray(inputs["mmean"], f32)
    mvar = np.asarray(inputs["mvar"], f32)
    Wp = np.asarray(inputs["Wp"], f32)
    bp = np.asarray(inputs["bp"], f32)

    def elu(v):
        return np.where(v > 0, v, np.expm1(v)).astype(f32)

    z = np.linspace(-1.0, 1.0, 21).astype(f32).reshape(-1, 1)
    h = elu(z @ W1 + b1)
    h = elu(h @ W2 + b2)
    h = (h - mmean) * (f32(1.0) / np.sqrt(mvar + f32(BN_EPS))) * gamma + beta
    o = h @ Wp + bp
    kout = (f32(1.0) / (f32(1.0) + np.exp(-o)))[:, 0]
    dL = kout[1:11] - kout[:10]
    dR = kout[11:] - kout[10:-1]
    pen = f32(0.5) * f32(PENALTY_RATE) * np.mean(
        np.abs(dL) - dL + np.abs(dR) - dR, dtype=f32
    )
    return f32(pen)


def kernel(**inputs):
    x = np.asarray(inputs["x"], np.float32)
    nc = build_program(inputs)
    out, _ = run_device(nc, x)
    penalty = host_penalty(inputs)
    return out, penalty


# revision 22
# speedup vs baseline: 31596.4905x; 31596.4905x over previous
"""Trainium2 Bass kernel for nn_FC_mono_12086037971055 (dense_mlp).

Computation (per batch b of x: (T=4096, C=256)):
  norm_x = x / sqrt(sum_t x^2 + 1e-7)          (column-normalize over T)
  cor    = norm_x^T @ norm_x                   (C x C Gram of correlations)
  att    = MLP(cor) elementwise                (1->4 elu ->4 elu -> BN -> 1 sigmoid)
  att    = att / (sum_axis1(att) + 1e-7)       (column-normalize)
  out    = x + x @ (offdiag * att)
plus a scalar monotonicity penalty from 21 keypoints (computed host-side: it
depends only on the tiny MLP params).

Sharding: data-parallel over batch B=32 -> 4 batches on each of 8 cores.
All MLP/BN parameters are folded on the host into scalar immediates that are
baked into the Bass program (BN is affine at inference, so it folds into the
final dense layer).

Kernel strategy per batch (one NeuronCore):
  - G = X^T X with raw X via PE matmul in float32r (full rate at N=256).
    Column norms Q_c are G's diagonal, extracted with an eye-mask reduce.
  - cor = s_c * s_d * G with s = 1/sqrt(Q+eps).  The per-free-dim scale is
    applied via PE transposes of row-scaled blocks (G is symmetric, so only
    blocks 00, 10, 11 are needed; 10 comes from transposing row-scaled 01).
  - MLP runs elementwise on one packed (128, 384) tile.
    elu(u) = min(exp(u) - 1, relu(u)), exp/relu/sigmoid on ScalarE.
  - att column-normalize: row-sums (symmetry) -> R = 1/(S+eps); the
    per-column application is again row-scale + PE transpose.  The offdiag
    mask and the +Identity fold (out = X @ (offdiag*att + I)) are applied
    during the PSUM->SBUF copies.
  - out rows: lhsT = X^T (built with PE transposes of X), rhs = A''.
"""

import numpy as np

import concourse.bass as bass
import concourse.tile as tile
from concourse import bacc, bass_utils, mybir

F32 = mybir.dt.float32
F32R = mybir.dt.float32r
F16 = mybir.dt.float16
AF = mybir.ActivationFunctionType
ALU = mybir.AluOpType
X_AX = mybir.AxisListType.X

N_CORES = 8
B_TOTAL = 32
B_PER_CORE = B_TOTAL // N_CORES   # 4
T = 4096
C = 256
P = 128
NT = T // P                        # 32 t-blocks
NCHUNK = 4                         # x streamed in 4 chunks per batch
LT = NT // NCHUNK                  # 8 t-blocks per chunk
BN_EPS = 1e-3
EPS = 1e-7
PENALTY_RATE = 10.0


def _fold_params(inputs):
    """Fold BN into the projection layer; return plain python floats."""
    W1 = np.asarray(inputs["W1"], np.float64)     # (1, 4)
    b1 = np.asarray(inputs["b1"], np.float64)     # (4,)
    W2 = np.asarray(inputs["W2"], np.float64)     # (4, 4)
    b2 = np.asarray(inputs["b2"], np.float64)     # (4,)
    gamma = np.asarray(inputs["gamma"], np.float64)
    beta = np.asarray(inputs["beta"], np.float64)
    mmean = np.asarray(inputs["mmean"], np.float64)
    mvar = np.asarray(inputs["mvar"], np.float64)
    Wp = np.asarray(inputs["Wp"], np.float64)     # (4, 1)
    bp = np.asarray(inputs["bp"], np.float64)     # (1,)

    a = gamma / np.sqrt(mvar + BN_EPS)
    wp2 = Wp[:, 0] * a
    bp2 = bp[0] + np.sum(Wp[:, 0] * (beta - mmean * a))
    return (
        [float(v) for v in W1[0]],
        [float(v) for v in b1],
        [[float(W2[i, j]) for j in range(4)] for i in range(4)],
        [float(v) for v in b2],
        [float(v) for v in wp2],
        float(bp2),
    )


class _St:
    pass


STAGE_RANGES = []


def _mark(nc, label):
    STAGE_RANGES.append((label, int(nc.next_id())))


def _build_kernel(tc, o_d, x_d, w1, b1, W2, b2, wp2, bp2):
    nc = tc.nc

    import contextlib
    ctx = contextlib.ExitStack()
    consts = ctx.enter_context(tc.tile_pool(name="consts", bufs=1))
    xin = ctx.enter_context(tc.tile_pool(name="xin", bufs=3))
    xrp = ctx.enter_context(tc.tile_pool(name="xrp", bufs=3))
    xtp = ctx.enter_context(tc.tile_pool(name="xtp", bufs=6))
    mlp = ctx.enter_context(tc.tile_pool(name="mlp", bufs=2))
    small = ctx.enter_context(tc.tile_pool(name="small", bufs=2))
    apool = ctx.enter_context(tc.tile_pool(name="apool", bufs=2))
    ostage = ctx.enter_context(tc.tile_pool(name="ostage", bufs=2))
    pp_gram = ctx.enter_context(tc.tile_pool(name="pp_gram", bufs=1, space="PSUM"))
    pp_xt = ctx.enter_context(tc.tile_pool(name="pp_xt", bufs=2, space="PSUM"))
    pp_mlp = ctx.enter_context(tc.tile_pool(name="pp_mlp", bufs=2, space="PSUM"))
    pp_out = ctx.enter_context(tc.tile_pool(name="pp_out", bufs=2, space="PSUM"))

    # constants: identity (also the eye mask) and offdiag = 1 - eye
    from concourse.masks import make_identity
    ident = consts.tile([P, P], F32, tag="ident")
    make_identity(nc, ident)
    identr = consts.tile([P, P], F32R, tag="identr")
    nc.vector.tensor_copy(identr, ident)
    od = consts.tile([P, P], F32, tag="od")
    nc.vector.tensor_scalar(
        out=od, in0=ident, scalar1=-1.0, scalar2=1.0, op0=ALU.mult, op1=ALU.add
    )

    # (128,1) constant tiles used as activation biases
    bias_tiles = {}

    def cbias(val):
        v = float(val)
        if v not in bias_tiles:
            t = consts.tile(
                [P, 1], F32, tag=f"cb{len(bias_tiles)}",
                name=f"cb{len(bias_tiles)}",
            )
            nc.vector.memset(t, v)
            bias_tiles[v] = t
        return bias_tiles[v]

    def r32(ap):
        return ap.bitcast(F32R)

    # ---------------- stage A: load, Gram, X^T, cor -> m_in ----------------
    def stage_a(b, st):
        # load x[b] in 4 chunks of (1024, 256)
        st.X = []
        for g in range(NCHUNK):
            xg = xin.tile([P, LT, C], F32, tag="xin")
            src = x_d[b, g * LT * P:(g + 1) * LT * P, :].rearrange(
                "(lt p) c -> p lt c", p=P
            )
            nc.sync.dma_start(out=xg, in_=src)
            # round to f32r for full-rate PE consumption (GPSIMD is idle)
            xr = xrp.tile([P, LT, C], F32R, tag="xr")
            nc.gpsimd.tensor_copy(xr, xg)
            st.X.append(xr)

        st.XT = [
            xtp.tile([P, T], F32R, tag="xt", name=f"xt_b{b}_c{cb}")
            for cb in range(2)
        ]

        G0 = pp_gram.tile([P, C], F32, tag="g0")
        G1 = pp_gram.tile([P, C], F32, tag="g1")

        for g in range(NCHUNK):
            xg = st.X[g]
            for lt in range(LT):
                k = g * LT + lt
                xa = xg[:, lt, :]
                nc.tensor.matmul(
                    G0[:, :], xa[:, 0:P], xa,
                    start=(k == 0), stop=(k == NT - 1),
                )
                nc.tensor.matmul(
                    G1[:, :], xa[:, P:C], xa,
                    start=(k == 0), stop=(k == NT - 1),
                )
            # transposes of this chunk into X^T
            for cb in range(2):
                for h in range(2):
                    tg = pp_xt.tile([P, 4 * P], F32R, tag="xtT")
                    for i in range(4):
                        lt = h * 4 + i
                        nc.tensor.transpose(
                            tg[:, i * P:(i + 1) * P],
                            xg[:, lt, cb * P:(cb + 1) * P],
                            identr,
                        )
                    t0 = (g * LT + h * 4) * P
                    dst = st.XT[cb][:, t0:t0 + 4 * P]
                    if (g * 2 + cb + h) % 2 == 0:
                        nc.scalar.copy(dst, tg)
                    else:
                        nc.vector.tensor_copy(dst, tg)

        # copy G out of PSUM immediately so the next batch's Gram can start
        Gs0 = small.tile([P, C], F32, tag="gs0")
        Gs1 = small.tile([P, P], F32, tag="gs1")
        nc.scalar.copy(Gs0, G0)
        nc.scalar.copy(Gs1, G1[:, P:C])

        # column norms from the Gram diagonal
        scr = small.tile([P, P], F32, tag="scr")
        Q0 = small.tile([P, 1], F32, tag="q0")
        Q1 = small.tile([P, 1], F32, tag="q1")
        nc.vector.tensor_mul(scr, Gs0[:, 0:P], ident)
        nc.vector.tensor_reduce(Q0, scr, axis=X_AX, op=ALU.add)
        scr2 = small.tile([P, P], F32, tag="scr2")
        nc.vector.tensor_mul(scr2, Gs1, ident)
        nc.vector.tensor_reduce(Q1, scr2, axis=X_AX, op=ALU.add)
        sq0 = small.tile([P, 1], F32, tag="sq0")
        sq1 = small.tile([P, 1], F32, tag="sq1")
        nc.scalar.activation(sq0, Q0, AF.Ln, bias=cbias(EPS), scale=1.0)
        nc.scalar.activation(sq1, Q1, AF.Ln, bias=cbias(EPS), scale=1.0)
        s0 = small.tile([P, 1], F32, tag="s0")
        s1 = small.tile([P, 1], F32, tag="s1")
        nc.scalar.activation(s0, sq0, AF.Exp, bias=cbias(0.0), scale=-0.5)
        nc.scalar.activation(s1, sq1, AF.Exp, bias=cbias(0.0), scale=-0.5)

        # cor blocks: row-scale, transpose, scale again on the copy out
        u00 = small.tile([P, P], F32, tag="u00")
        u01 = small.tile([P, P], F32, tag="u01")
        u11 = small.tile([P, P], F32, tag="u11")
        nc.vector.tensor_scalar_mul(u00, Gs0[:, 0:P], s0)
        nc.vector.tensor_scalar_mul(u01, Gs0[:, P:C], s0)
        nc.vector.tensor_scalar_mul(u11, Gs1, s1)
        tgc = pp_mlp.tile([P, 3 * P], F32, tag="mlpT")
        nc.tensor.transpose(tgc[:, 0:P], u00, ident)
        nc.tensor.transpose(tgc[:, P:2 * P], u01, ident)
        nc.tensor.transpose(tgc[:, 2 * P:3 * P], u11, ident)
        m_in = mlp.tile([P, 3 * P], F32, tag="m_in", bufs=3)
        nc.vector.tensor_scalar_mul(m_in[:, 0:P], tgc[:, 0:P], s0)
        nc.vector.tensor_scalar_mul(m_in[:, P:2 * P], tgc[:, P:2 * P], s1)
        nc.vector.tensor_scalar_mul(m_in[:, 2 * P:3 * P], tgc[:, 2 * P:3 * P], s1)
        st.m_in = m_in

    # ---------------- stage B: MLP + normalize -> A'' ----------------
    def stage_b(b, st):
        m_in = st.m_in
        W = 3 * P

        def elu_pair(src, scale, bias):
            E = mlp.tile([P, W], F32, tag="e")
            R = mlp.tile([P, W], F32, tag="r")
            nc.scalar.activation(E, src, AF.Exp, bias=bias, scale=scale)
            nc.scalar.activation(R, src, AF.Relu, bias=bias, scale=scale)
            h = mlp.tile([P, W], F32, tag="h")
            nc.vector.scalar_tensor_tensor(
                h, in0=E, scalar=-1.0, in1=R, op0=ALU.add, op1=ALU.min
            )
            return h

        H1 = []
        for i in range(4):
            E = mlp.tile([P, W], F16, tag="e")
            R = mlp.tile([P, W], F16, tag="r")
            nc.scalar.activation(E, m_in, AF.Exp, bias=cbias(b1[i]), scale=w1[i])
            import os as _os
            if b1[i] == 0.0 and _os.environ.get("K_R1", "act") == "dve":
                nc.vector.tensor_scalar(
                    out=R, in0=m_in, scalar1=w1[i], scalar2=0.0,
                    op0=ALU.mult, op1=ALU.max,
                )
            else:
                nc.scalar.activation(R, m_in, AF.Relu, bias=cbias(b1[i]), scale=w1[i])
            h = mlp.tile([P, W], F16, tag=f"h1_{i}", bufs=1)
            nc.vector.scalar_tensor_tensor(
                h, in0=E, scalar=-1.0, in1=R, op0=ALU.add, op1=ALU.min
            )
            H1.append(h)

        H2 = []
        for j in range(4):
            import os as _os
            _l2g = _os.environ.get("K_L2G", "dve")
            if _l2g == "odd":
                eng = nc.gpsimd if j % 2 == 1 else nc.vector
            elif _l2g == "j3":
                eng = nc.gpsimd if j == 3 else nc.vector
            else:
                eng = nc.vector
            cur = mlp.tile([P, W], F16, tag="u")
            eng.tensor_scalar(
                out=cur, in0=H1[0], scalar1=W2[0][j], scalar2=b2[j],
                op0=ALU.mult, op1=ALU.add,
            )
            for i in range(1, 4):
                nxt = mlp.tile([P, W], F16, tag="u")
                eng.scalar_tensor_tensor(
                    nxt, in0=H1[i], scalar=W2[i][j], in1=cur,
                    op0=ALU.mult, op1=ALU.add,
                )
                cur = nxt
            E = mlp.tile([P, W], F16, tag="e")
            R = mlp.tile([P, W], F16, tag="r")
            nc.scalar.activation(E, cur, AF.Exp, bias=cbias(0.0))
            nc.scalar.activation(R, cur, AF.Relu, bias=cbias(0.0))
            h = mlp.tile([P, W], F16, tag=f"h2_{j}", bufs=1)
            nc.vector.scalar_tensor_tensor(
                h, in0=E, scalar=-1.0, in1=R, op0=ALU.add, op1=ALU.min
            )
            H2.append(h)

        import os as _os
        _l3 = nc.gpsimd if _os.environ.get("K_L3", "dve") == "gps" else nc.vector
        cur = mlp.tile([P, W], F16, tag="u")
        _l3.tensor_scalar(
            out=cur, in0=H2[0], scalar1=wp2[0], scalar2=bp2,
            op0=ALU.mult, op1=ALU.add,
        )
        for j in range(1, 4):
            nxt = mlp.tile([P, W], F16, tag="u")
            _l3.scalar_tensor_tensor(
                nxt, in0=H2[j], scalar=wp2[j], in1=cur, op0=ALU.mult, op1=ALU.add
            )
            cur = nxt
        eneg = mlp.tile([P, W], F16, tag="e")
        nc.scalar.activation(eneg, cur, AF.Exp, bias=cbias(0.0), scale=-1.0)
        wden = mlp.tile([P, W], F32, tag="u")
        nc.vector.tensor_scalar_add(wden, eneg, 1.0)
        a_out = mlp.tile([P, W], F32, tag="a_out")
        nc.vector.reciprocal(a_out, wden)

        # A01 = A10^T  (att pre-normalization is symmetric)
        t01p = pp_mlp.tile([P, 3 * P], F32, tag="mlpT")
        nc.tensor.transpose(t01p[:, 0:P], a_out[:, P:2 * P], ident)
        t01 = small.tile([P, P], F32, tag="t01")
        S01 = small.tile([P, 1], F32, tag="s01")
        nc.scalar.copy(t01, t01p[:, 0:P])
        nc.vector.tensor_reduce(S01, t01, axis=X_AX, op=ALU.add)

        # column sums via row sums (symmetry)
        Sa = small.tile([P, 1], F32, tag="sa")
        nc.vector.tensor_reduce(Sa, a_out[:, 0:P], axis=X_AX, op=ALU.add)
        Sc0 = small.tile([P, 1], F32, tag="sc0")
        nc.vector.tensor_add(Sc0, Sa, S01)
        Sc1 = small.tile([P, 1], F32, tag="sc1")
        nc.vector.tensor_reduce(Sc1, a_out[:, P:3 * P], axis=X_AX, op=ALU.add)
        R0 = small.tile([P, 1], F32, tag="r0")
        R1 = small.tile([P, 1], F32, tag="r1")
        t0 = small.tile([P, 1], F32, tag="t0")
        t1 = small.tile([P, 1], F32, tag="t1")
        nc.vector.tensor_scalar_add(t0, Sc0, EPS)
        nc.vector.tensor_scalar_add(t1, Sc1, EPS)
        nc.vector.reciprocal(R0, t0)
        nc.vector.reciprocal(R1, t1)

        # U = R * (A * offdiag-mask), blockwise
        n00 = small.tile([P, P], F32, tag="n00")
        n11 = small.tile([P, P], F32, tag="n11")
        nc.vector.tensor_mul(n00, a_out[:, 0:P], od)
        nc.vector.tensor_mul(n11, a_out[:, 2 * P:3 * P], od)
        v00 = small.tile([P, P], F32, tag="v00")
        v01 = small.tile([P, P], F32, tag="v01")
        v10 = small.tile([P, P], F32, tag="v10")
        v11 = small.tile([P, P], F32, tag="v11")
        nc.vector.tensor_scalar_mul(v00, n00, R0)
        nc.vector.tensor_scalar_mul(v01, t01, R0)
        nc.vector.tensor_scalar_mul(v10, a_out[:, P:2 * P], R1)
        nc.vector.tensor_scalar_mul(v11, n11, R1)

        # A'' = U^T + I
        p0 = pp_mlp.tile([P, 3 * P], F32, tag="mlpT")
        nc.tensor.transpose(p0[:, 0:P], v00, ident)
        nc.tensor.transpose(p0[:, P:2 * P], v10, ident)
        p1 = pp_mlp.tile([P, 3 * P], F32, tag="mlpT")
        nc.tensor.transpose(p1[:, 0:P], v01, ident)
        nc.tensor.transpose(p1[:, P:2 * P], v11, ident)
        A0 = apool.tile([P, C], F32R, tag="A0")
        A1 = apool.tile([P, C], F32R, tag="A1")
        nc.vector.scalar_tensor_tensor(
            A0[:, 0:P], in0=p0[:, 0:P], scalar=1.0, in1=ident,
            op0=ALU.mult, op1=ALU.add,
        )
        nc.scalar.copy(A0[:, P:C], p0[:, P:2 * P])
        nc.scalar.copy(A1[:, 0:P], p1[:, 0:P])
        nc.vector.scalar_tensor_tensor(
            A1[:, P:C], in0=p1[:, P:2 * P], scalar=1.0, in1=ident,
            op0=ALU.mult, op1=ALU.add,
        )
        st.A = (A0, A1)

    # ---------------- stage C: out = X @ A'' ----------------
    def stage_c(b, st):
        A0, A1 = st.A
        XT0, XT1 = st.XT
        for g in range(NCHUNK):
            ost = ostage.tile([P, LT, C], F32, tag="ost")
            for lt in range(LT):
                tb = g * LT + lt
                po = pp_out.tile([P, C], F32, tag="po")
                nc.tensor.matmul(
                    po, XT0[:, tb * P:(tb + 1) * P], A0,
                    start=True, stop=False,
                )
                nc.tensor.matmul(
                    po, XT1[:, tb * P:(tb + 1) * P], A1,
                    start=False, stop=True,
                )
                if lt % 2 == 0:
                    nc.scalar.copy(ost[:, lt, :], po)
                else:
                    nc.vector.tensor_copy(ost[:, lt, :], po)
            dst = o_d[b, g * LT * P:(g + 1) * LT * P, :].rearrange(
                "(lt p) c -> p lt c", p=P
            )
            nc.sync.dma_start(out=dst, in_=ost)

    # ---------------- software-pipelined emission ----------------
    import os as _os
    nb = int(_os.environ.get("KERNEL_NBATCH", str(B_PER_CORE)))
    _ = _os
    sts = [_St() for _ in range(B_PER_CORE)]
    del STAGE_RANGES[:]
    _stage_a, _stage_b, _stage_c = stage_a, stage_b, stage_c

    def stage_a(b, st):
        _mark(nc, f"A{b}")
        _stage_a(b, st)

    def stage_b(b, st):
        _mark(nc, f"B{b}")
        _stage_b(b, st)

    def stage_c(b, st):
        _mark(nc, f"C{b}")
        _stage_c(b, st)

    if nb == 1:
        stage_a(0, sts[0]); stage_b(0, sts[0]); stage_c(0, sts[0])
    elif nb == 2:
        stage_a(0, sts[0]); stage_a(1, sts[1])
        stage_b(0, sts[0]); stage_c(0, sts[0])
        stage_b(1, sts[1]); stage_c(1, sts[1])
    elif _os.environ.get("K_PIPE", "3") == "3":
        stage_a(0, sts[0])
        stage_a(1, sts[1])
        stage_b(0, sts[0])
        stage_a(2, sts[2])
        stage_c(0, sts[0])
        stage_b(1, sts[1])
        stage_a(3, sts[3])
        stage_c(1, sts[1])
        stage_b(2, sts[2])
        stage_c(2, sts[2])
        stage_b(3, sts[3])
        stage_c(3, sts[3])
    else:
        stage_a(0, sts[0])
        stage_a(1, sts[1])
        stage_b(0, sts[0])
        stage_c(0, sts[0])
        stage_a(2, sts[2])
        stage_b(1, sts[1])
        stage_c(1, sts[1])
        stage_a(3, sts[3])
        stage_b(2, sts[2])
        stage_c(2, sts[2])
        stage_b(3, sts[3])
        stage_c(3, sts[3])

    ctx.close()


_ACT_TABLES_PATCHED = False


def _pin_act_table():
    """Force the act-table chooser onto natural_log_exp_and_others (which
    covers Copy/Ln/Exp/Relu) so the whole kernel needs ONE table load
    instead of thrashing between exp_and_others and natural_log."""
    global _ACT_TABLES_PATCHED
    if _ACT_TABLES_PATCHED:
        return
    from concourse import hw_specs
    import concourse.bacc as bacc_mod
    orig = hw_specs.get_activation_tables
    mine = {AF.Copy, AF.Ln, AF.Exp, AF.Relu, AF.Identity}
    keep = "natural_log_exp_and_others"

    def patched(arch):
        tabs = orig(arch)
        if keep not in tabs or not mine <= tabs[keep]:
            return tabs
        return {
            name: (s if name == keep else s - mine)
            for name, s in tabs.items()
        }

    bacc_mod.get_activation_tables = patched
    _ACT_TABLES_PATCHED = True


def build_program(inputs):
    """Build + compile the SPMD Bass program with folded params baked in."""
    _pin_act_table()
    w1, b1, W2, b2, wp2, bp2 = _fold_params(inputs)
    nc = bacc.Bacc(
        "TRN2",
        target_bir_lowering=False,
        debug=False,
        enable_asserts=False,
        num_devices=N_CORES,
    )
    x_d = nc.dram_tensor("x", (B_PER_CORE, T, C), F32, kind="ExternalInput").ap()
    o_d = nc.dram_tensor("out", (B_PER_CORE, T, C), F32, kind="ExternalOutput").ap()
    with tile.TileContext(nc) as tc:
        _build_kernel(tc, o_d, x_d, w1, b1, W2, b2, wp2, bp2)
    nc.compile()
    return nc


def run_device(nc, x, trace=False, **kw):
    """Run the compiled program over the 8 cores; return (out, results)."""
    x = np.asarray(x, np.float32)
    in_maps = [
        {"x": np.ascontiguousarray(x[c * B_PER_CORE:(c + 1) * B_PER_CORE])}
        for c in range(N_CORES)
    ]
    res = bass_utils.run_bass_kernel_spmd(
        nc, in_maps, core_ids=list(range(N_CORES)), trace=trace, **kw
    )
    out = np.concatenate(
        [res.results[c]["out"] for c in range(N_CORES)], axis=0
    )
    return out, res


def host_penalty(inputs):
    """Monotonicity penalty on 21 keypoints; float32 math mirroring reference."""
    f32 = np.float32
    W1 = np.asarray(inputs["W1"], f32)
    b1 = np.asarray(inputs["b1"], f32)
    W2 = np.asarray(inputs["W2"], f32)
    b2 = np.asarray(inputs["b2"], f32)
    gamma = np.asarray(inputs["gamma"], f32)
    beta = np.asarray(inputs["beta"], f32)
    mmean = np.asarray(inputs["mmean"], f32)
    mvar = np.asarray(inputs["mvar"], f32)
    Wp = np.asarray(inputs["Wp"], f32)
    bp = np.asarray(inputs["bp"], f32)

    def elu(v):
        return np.where(v > 0, v, np.expm1(v)).astype(f32)

    z = np.linspace(-1.0, 1.0, 21).astype(f32).reshape(-1, 1)
    h = elu(z @ W1 + b1)
    h = elu(h @ W2 + b2)
    h = (h - mmean) * (f32(1.0) / np.sqrt(mvar + f32(BN_EPS))) * gamma + beta
    o = h @ Wp + bp
    kout = (f32(1.0) / (f32(1.0) + np.exp(-o)))[:, 0]
    dL = kout[1:11] - kout[:10]
    dR = kout[11:] - kout[10:-1]
    pen = f32(0.5) * f32(PENALTY_RATE) * np.mean(
        np.abs(dL) - dL + np.abs(dR) - dR, dtype=f32
    )
    return f32(pen)


def kernel(**inputs):
    x = np.asarray(inputs["x"], np.float32)
    nc = build_program(inputs)
    out, _ = run_device(nc, x)
    penalty = host_penalty(inputs)
    return out, penalty


# revision 26
# speedup vs baseline: 33560.3460x; 1.0622x over previous
"""Trainium2 Bass kernel for nn_FC_mono_12086037971055 (dense_mlp).

Computation (per batch b of x: (T=4096, C=256)):
  norm_x = x / sqrt(sum_t x^2 + 1e-7)          (column-normalize over T)
  cor    = norm_x^T @ norm_x                   (C x C Gram of correlations)
  att    = MLP(cor) elementwise                (1->4 elu ->4 elu -> BN -> 1 sigmoid)
  att    = att / (sum_axis1(att) + 1e-7)       (column-normalize)
  out    = x + x @ (offdiag * att)
plus a scalar monotonicity penalty from 21 keypoints (computed host-side: it
depends only on the tiny MLP params).

Sharding: data-parallel over batch B=32 -> 4 batches on each of 8 cores.
All MLP/BN parameters are folded on the host into scalar immediates that are
baked into the Bass program (BN is affine at inference, so it folds into the
final dense layer).

Kernel strategy per batch (one NeuronCore):
  - G = X^T X with raw X via PE matmul in float32r (full rate at N=256).
    Column norms Q_c are G's diagonal, extracted with an eye-mask reduce.
  - cor = s_c * s_d * G with s = 1/sqrt(Q+eps).  The per-free-dim scale is
    applied via PE transposes of row-scaled blocks (G is symmetric, so only
    blocks 00, 10, 11 are needed; 10 comes from transposing row-scaled 01).
  - MLP runs elementwise on one packed (128, 384) tile.
    elu(u) = min(exp(u) - 1, relu(u)), exp/relu/sigmoid on ScalarE.
  - att column-normalize: row-sums (symmetry) -> R = 1/(S+eps); the
    per-column application is again row-scale + PE transpose.  The offdiag
    mask and the +Identity fold (out = X @ (offdiag*att + I)) are applied
    during the PSUM->SBUF copies.
  - out rows: lhsT = X^T (built with PE transposes of X), rhs = A''.
"""

import numpy as np

import concourse.bass as bass
import concourse.tile as tile
from concourse import bacc, bass_utils, mybir

F32 = mybir.dt.float32
F32R = mybir.dt.float32r
F16 = mybir.dt.float16
AF = mybir.ActivationFunctionType
ALU = mybir.AluOpType
X_AX = mybir.AxisListType.X

N_CORES = 8
B_TOTAL = 32
B_PER_CORE = B_TOTAL // N_CORES   # 4
T = 4096
C = 256
P = 128
NT = T // P                        # 32 t-blocks
NCHUNK = 8                         # x streamed in 8 chunks per batch
LT = NT // NCHUNK                  # 8 t-blocks per chunk
BN_EPS = 1e-3
EPS = 1e-7
PENALTY_RATE = 10.0


def _fold_params(inputs):
    """Fold BN into the projection layer; return plain python floats."""
    W1 = np.asarray(inputs["W1"], np.float64)     # (1, 4)
    b1 = np.asarray(inputs["b1"], np.float64)     # (4,)
    W2 = np.asarray(inputs["W2"], np.float64)     # (4, 4)
    b2 = np.asarray(inputs["b2"], np.float64)     # (4,)
    gamma = np.asarray(inputs["gamma"], np.float64)
    beta = np.asarray(inputs["beta"], np.float64)
    mmean = np.asarray(inputs["mmean"], np.float64)
    mvar = np.asarray(inputs["mvar"], np.float64)
    Wp = np.asarray(inputs["Wp"], np.float64)     # (4, 1)
    bp = np.asarray(inputs["bp"], np.float64)     # (1,)

    a = gamma / np.sqrt(mvar + BN_EPS)
    wp2 = Wp[:, 0] * a
    bp2 = bp[0] + np.sum(Wp[:, 0] * (beta - mmean * a))
    return (
        [float(v) for v in W1[0]],
        [float(v) for v in b1],
        [[float(W2[i, j]) for j in range(4)] for i in range(4)],
        [float(v) for v in b2],
        [float(v) for v in wp2],
        float(bp2),
    )


class _St:
    pass


STAGE_RANGES = []


def _mark(nc, label):
    STAGE_RANGES.append((label, int(nc.next_id())))


def _build_kernel(tc, o_d, x_d, w1, b1, W2, b2, wp2, bp2):
    nc = tc.nc

    import contextlib
    ctx = contextlib.ExitStack()
    consts = ctx.enter_context(tc.tile_pool(name="consts", bufs=1))
    xin = ctx.enter_context(tc.tile_pool(name="xin", bufs=6))
    xrp = ctx.enter_context(tc.tile_pool(name="xrp", bufs=6))
    xtp = ctx.enter_context(tc.tile_pool(name="xtp", bufs=6))
    mlp = ctx.enter_context(tc.tile_pool(name="mlp", bufs=2))
    small = ctx.enter_context(tc.tile_pool(name="small", bufs=2))
    apool = ctx.enter_context(tc.tile_pool(name="apool", bufs=2))
    ostage = ctx.enter_context(tc.tile_pool(name="ostage", bufs=4))
    pp_gram = ctx.enter_context(tc.tile_pool(name="pp_gram", bufs=1, space="PSUM"))
    pp_xt = ctx.enter_context(tc.tile_pool(name="pp_xt", bufs=2, space="PSUM"))
    pp_mlp = ctx.enter_context(tc.tile_pool(name="pp_mlp", bufs=2, space="PSUM"))
    pp_out = ctx.enter_context(tc.tile_pool(name="pp_out", bufs=2, space="PSUM"))

    # constants: identity (also the eye mask) and offdiag = 1 - eye
    from concourse.masks import make_identity
    ident = consts.tile([P, P], F32, tag="ident")
    make_identity(nc, ident)
    identr = consts.tile([P, P], F32R, tag="identr")
    nc.vector.tensor_copy(identr, ident)
    od = consts.tile([P, P], F32, tag="od")
    nc.vector.tensor_scalar(
        out=od, in0=ident, scalar1=-1.0, scalar2=1.0, op0=ALU.mult, op1=ALU.add
    )

    # (128,1) constant tiles used as activation biases
    bias_tiles = {}

    def cbias(val):
        v = float(val)
        if v not in bias_tiles:
            t = consts.tile(
                [P, 1], F32, tag=f"cb{len(bias_tiles)}",
                name=f"cb{len(bias_tiles)}",
            )
            nc.vector.memset(t, v)
            bias_tiles[v] = t
        return bias_tiles[v]

    def r32(ap):
        return ap.bitcast(F32R)

    # ---------------- stage A: load, Gram, X^T, cor -> m_in ----------------
    def stage_a(b, st):
        # load x[b] in 4 chunks of (1024, 256)
        st.X = []
        for g in range(NCHUNK):
            xg = xin.tile([P, LT, C], F32, tag="xin")
            src = x_d[b, g * LT * P:(g + 1) * LT * P, :].rearrange(
                "(lt p) c -> p lt c", p=P
            )
            nc.sync.dma_start(out=xg, in_=src)
            # round to f32r for full-rate PE consumption (GPSIMD is idle)
            xr = xrp.tile([P, LT, C], F32R, tag="xr")
            nc.gpsimd.tensor_copy(xr, xg)
            st.X.append(xr)

        st.XT = [
            xtp.tile([P, T], F32R, tag="xt", name=f"xt_b{b}_c{cb}")
            for cb in range(2)
        ]

        G0 = pp_gram.tile([P, C], F32, tag="g0")
        G1 = pp_gram.tile([P, C], F32, tag="g1")

        for g in range(NCHUNK):
            yield
            xg = st.X[g]
            for lt in range(LT):
                k = g * LT + lt
                xa = xg[:, lt, :]
                nc.tensor.matmul(
                    G0[:, :], xa[:, 0:P], xa,
                    start=(k == 0), stop=(k == NT - 1),
                )
                nc.tensor.matmul(
                    G1[:, :], xa[:, P:C], xa,
                    start=(k == 0), stop=(k == NT - 1),
                )
            # transposes of this chunk into X^T
            for cb in range(2):
                for h in range(LT // 4):
                    tg = pp_xt.tile([P, 4 * P], F32R, tag="xtT")
                    for i in range(4):
                        lt = h * 4 + i
                        nc.tensor.transpose(
                            tg[:, i * P:(i + 1) * P],
                            xg[:, lt, cb * P:(cb + 1) * P],
                            identr,
                        )
                    t0 = (g * LT + h * 4) * P
                    dst = st.XT[cb][:, t0:t0 + 4 * P]
                    if (g * 2 + cb + h) % 4 != 3:
                        nc.scalar.copy(dst, tg)
                    else:
                        nc.vector.tensor_copy(dst, tg)

        # copy G out of PSUM immediately so the next batch's Gram can start
        Gs0 = small.tile([P, C], F32, tag="gs0")
        Gs1 = small.tile([P, P], F32, tag="gs1")
        nc.scalar.copy(Gs0, G0)
        nc.scalar.copy(Gs1, G1[:, P:C])

        # column norms from the Gram diagonal
        scr = small.tile([P, P], F32, tag="scr")
        Q0 = small.tile([P, 1], F32, tag="q0")
        Q1 = small.tile([P, 1], F32, tag="q1")
        nc.vector.tensor_mul(scr, Gs0[:, 0:P], ident)
        nc.vector.tensor_reduce(Q0, scr, axis=X_AX, op=ALU.add)
        scr2 = small.tile([P, P], F32, tag="scr2")
        nc.vector.tensor_mul(scr2, Gs1, ident)
        nc.vector.tensor_reduce(Q1, scr2, axis=X_AX, op=ALU.add)
        sq0 = small.tile([P, 1], F32, tag="sq0")
        sq1 = small.tile([P, 1], F32, tag="sq1")
        nc.scalar.activation(sq0, Q0, AF.Ln, bias=cbias(EPS), scale=1.0)
        nc.scalar.activation(sq1, Q1, AF.Ln, bias=cbias(EPS), scale=1.0)
        s0 = small.tile([P, 1], F32, tag="s0")
        s1 = small.tile([P, 1], F32, tag="s1")
        nc.scalar.activation(s0, sq0, AF.Exp, bias=cbias(0.0), scale=-0.5)
        nc.scalar.activation(s1, sq1, AF.Exp, bias=cbias(0.0), scale=-0.5)

        # cor blocks: row-scale, transpose, scale again on the copy out
        u00 = small.tile([P, P], F32, tag="u00")
        u01 = small.tile([P, P], F32, tag="u01")
        u11 = small.tile([P, P], F32, tag="u11")
        nc.vector.tensor_scalar_mul(u00, Gs0[:, 0:P], s0)
        nc.vector.tensor_scalar_mul(u01, Gs0[:, P:C], s0)
        nc.vector.tensor_scalar_mul(u11, Gs1, s1)
        tgc = pp_mlp.tile([P, 3 * P], F32, tag="mlpT")
        nc.tensor.transpose(tgc[:, 0:P], u00, ident)
        nc.tensor.transpose(tgc[:, P:2 * P], u01, ident)
        nc.tensor.transpose(tgc[:, 2 * P:3 * P], u11, ident)
        m_in = mlp.tile([P, 3 * P], F32, tag="m_in", bufs=3)
        nc.vector.tensor_scalar_mul(m_in[:, 0:P], tgc[:, 0:P], s0)
        nc.vector.tensor_scalar_mul(m_in[:, P:2 * P], tgc[:, P:2 * P], s1)
        nc.vector.tensor_scalar_mul(m_in[:, 2 * P:3 * P], tgc[:, 2 * P:3 * P], s1)
        st.m_in = m_in

    # ---------------- stage B: MLP + normalize -> A'' ----------------
    def stage_b(b, st):
        m_in = st.m_in
        W = 3 * P

        def elu_pair(src, scale, bias):
            E = mlp.tile([P, W], F32, tag="e")
            R = mlp.tile([P, W], F32, tag="r")
            nc.scalar.activation(E, src, AF.Exp, bias=bias, scale=scale)
            nc.scalar.activation(R, src, AF.Relu, bias=bias, scale=scale)
            h = mlp.tile([P, W], F32, tag="h")
            nc.vector.scalar_tensor_tensor(
                h, in0=E, scalar=-1.0, in1=R, op0=ALU.add, op1=ALU.min
            )
            return h

        H1 = []
        for i in range(4):
            E = mlp.tile([P, W], F16, tag="e")
            R = mlp.tile([P, W], F16, tag="r")
            nc.scalar.activation(E, m_in, AF.Exp, bias=cbias(b1[i]), scale=w1[i])
            import os as _os
            if b1[i] == 0.0 and _os.environ.get("K_R1", "act") == "dve":
                nc.vector.tensor_scalar(
                    out=R, in0=m_in, scalar1=w1[i], scalar2=0.0,
                    op0=ALU.mult, op1=ALU.max,
                )
            else:
                nc.scalar.activation(R, m_in, AF.Relu, bias=cbias(b1[i]), scale=w1[i])
            h = mlp.tile([P, W], F16, tag=f"h1_{i}", bufs=1)
            nc.vector.scalar_tensor_tensor(
                h, in0=E, scalar=-1.0, in1=R, op0=ALU.add, op1=ALU.min
            )
            H1.append(h)
            if i % 2 == 1:
                yield

        H2 = []
        for j in range(4):
            import os as _os
            _l2g = _os.environ.get("K_L2G", "dve")
            if _l2g == "odd":
                eng = nc.gpsimd if j % 2 == 1 else nc.vector
            elif _l2g == "j3":
                eng = nc.gpsimd if j == 3 else nc.vector
            else:
                eng = nc.vector
            cur = mlp.tile([P, W], F16, tag="u")
            eng.tensor_scalar(
                out=cur, in0=H1[0], scalar1=W2[0][j], scalar2=b2[j],
                op0=ALU.mult, op1=ALU.add,
            )
            for i in range(1, 4):
                nxt = mlp.tile([P, W], F16, tag="u")
                eng.scalar_tensor_tensor(
                    nxt, in0=H1[i], scalar=W2[i][j], in1=cur,
                    op0=ALU.mult, op1=ALU.add,
                )
                cur = nxt
            E = mlp.tile([P, W], F16, tag="e")
            R = mlp.tile([P, W], F16, tag="r")
            nc.scalar.activation(E, cur, AF.Exp, bias=cbias(0.0))
            nc.scalar.activation(R, cur, AF.Relu, bias=cbias(0.0))
            h = mlp.tile([P, W], F16, tag=f"h2_{j}", bufs=1)
            nc.vector.scalar_tensor_tensor(
                h, in0=E, scalar=-1.0, in1=R, op0=ALU.add, op1=ALU.min
            )
            H2.append(h)
            yield

        import os as _os
        _l3 = nc.gpsimd if _os.environ.get("K_L3", "dve") == "gps" else nc.vector
        cur = mlp.tile([P, W], F16, tag="u")
        _l3.tensor_scalar(
            out=cur, in0=H2[0], scalar1=wp2[0], scalar2=bp2,
            op0=ALU.mult, op1=ALU.add,
        )
        for j in range(1, 4):
            nxt = mlp.tile([P, W], F16, tag="u")
            _l3.scalar_tensor_tensor(
                nxt, in0=H2[j], scalar=wp2[j], in1=cur, op0=ALU.mult, op1=ALU.add
            )
            cur = nxt
        eneg = mlp.tile([P, W], F16, tag="e")
        nc.scalar.activation(eneg, cur, AF.Exp, bias=cbias(0.0), scale=-1.0)
        wden = mlp.tile([P, W], F32, tag="u")
        nc.vector.tensor_scalar_add(wden, eneg, 1.0)
        a_out = mlp.tile([P, W], F32, tag="a_out")
        nc.vector.reciprocal(a_out, wden)
        yield

        # A01 = A10^T  (att pre-normalization is symmetric)
        t01p = pp_mlp.tile([P, 3 * P], F32, tag="mlpT")
        nc.tensor.transpose(t01p[:, 0:P], a_out[:, P:2 * P], ident)
        t01 = small.tile([P, P], F32, tag="t01")
        S01 = small.tile([P, 1], F32, tag="s01")
        nc.scalar.copy(t01, t01p[:, 0:P])
        nc.vector.tensor_reduce(S01, t01, axis=X_AX, op=ALU.add)

        # column sums via row sums (symmetry)
        Sa = small.tile([P, 1], F32, tag="sa")
        nc.vector.tensor_reduce(Sa, a_out[:, 0:P], axis=X_AX, op=ALU.add)
        Sc0 = small.tile([P, 1], F32, tag="sc0")
        nc.vector.tensor_add(Sc0, Sa, S01)
        Sc1 = small.tile([P, 1], F32, tag="sc1")
        nc.vector.tensor_reduce(Sc1, a_out[:, P:3 * P], axis=X_AX, op=ALU.add)
        yield
        R0 = small.tile([P, 1], F32, tag="r0")
        R1 = small.tile([P, 1], F32, tag="r1")
        t0 = small.tile([P, 1], F32, tag="t0")
        t1 = small.tile([P, 1], F32, tag="t1")
        nc.vector.tensor_scalar_add(t0, Sc0, EPS)
        nc.vector.tensor_scalar_add(t1, Sc1, EPS)
        nc.vector.reciprocal(R0, t0)
        nc.vector.reciprocal(R1, t1)

        # U = R * (A * offdiag-mask), blockwise
        n00 = small.tile([P, P], F32, tag="n00")
        n11 = small.tile([P, P], F32, tag="n11")
        nc.vector.tensor_mul(n00, a_out[:, 0:P], od)
        nc.vector.tensor_mul(n11, a_out[:, 2 * P:3 * P], od)
        v00 = small.tile([P, P], F32, tag="v00")
        v01 = small.tile([P, P], F32, tag="v01")
        v10 = small.tile([P, P], F32, tag="v10")
        v11 = small.tile([P, P], F32, tag="v11")
        nc.vector.tensor_scalar_mul(v00, n00, R0)
        nc.vector.tensor_scalar_mul(v01, t01, R0)
        nc.vector.tensor_scalar_mul(v10, a_out[:, P:2 * P], R1)
        nc.vector.tensor_scalar_mul(v11, n11, R1)

        # A'' = U^T + I
        p0 = pp_mlp.tile([P, 3 * P], F32, tag="mlpT")
        nc.tensor.transpose(p0[:, 0:P], v00, ident)
        nc.tensor.transpose(p0[:, P:2 * P], v10, ident)
        p1 = pp_mlp.tile([P, 3 * P], F32, tag="mlpT")
        nc.tensor.transpose(p1[:, 0:P], v01, ident)
        nc.tensor.transpose(p1[:, P:2 * P], v11, ident)
        A0 = apool.tile([P, C], F32R, tag="A0")
        A1 = apool.tile([P, C], F32R, tag="A1")
        nc.vector.scalar_tensor_tensor(
            A0[:, 0:P], in0=p0[:, 0:P], scalar=1.0, in1=ident,
            op0=ALU.mult, op1=ALU.add,
        )
        nc.scalar.copy(A0[:, P:C], p0[:, P:2 * P])
        nc.scalar.copy(A1[:, 0:P], p1[:, 0:P])
        nc.vector.scalar_tensor_tensor(
            A1[:, P:C], in0=p1[:, P:2 * P], scalar=1.0, in1=ident,
            op0=ALU.mult, op1=ALU.add,
        )
        st.A = (A0, A1)

    # ---------------- stage C: out = X @ A'' ----------------
    def stage_c(b, st):
        A0, A1 = st.A
        XT0, XT1 = st.XT
        for g in range(NCHUNK):
            ost = ostage.tile([P, LT, C], F32, tag="ost")
            for lt in range(LT):
                yield
                tb = g * LT + lt
                po = pp_out.tile([P, C], F32, tag="po")
                nc.tensor.matmul(
                    po, XT0[:, tb * P:(tb + 1) * P], A0,
                    start=True, stop=False,
                )
                nc.tensor.matmul(
                    po, XT1[:, tb * P:(tb + 1) * P], A1,
                    start=False, stop=True,
                )
                if lt % 2 == 0:
                    nc.scalar.copy(ost[:, lt, :], po)
                else:
                    nc.vector.tensor_copy(ost[:, lt, :], po)
            dst = o_d[b, g * LT * P:(g + 1) * LT * P, :].rearrange(
                "(lt p) c -> p lt c", p=P
            )
            nc.sync.dma_start(out=dst, in_=ost)

    # ---------------- software-pipelined emission ----------------
    import os as _os
    nb = int(_os.environ.get("KERNEL_NBATCH", str(B_PER_CORE)))
    _ = _os
    sts = [_St() for _ in range(B_PER_CORE)]
    del STAGE_RANGES[:]
    _ga, _gb, _gc = stage_a, stage_b, stage_c

    def drive(*gens):
        live = list(gens)
        while live:
            for g in list(live):
                try:
                    next(g)
                except StopIteration:
                    live.remove(g)

    def _tag(label, gen):
        _mark(nc, label)
        return gen

    def stage_a(b, st):
        _mark(nc, f"A{b}")
        drive(_ga(b, st))

    def stage_b(b, st):
        _mark(nc, f"B{b}")
        drive(_gb(b, st))

    def stage_c(b, st):
        _mark(nc, f"C{b}")
        drive(_gc(b, st))

    if nb == 1:
        stage_a(0, sts[0]); stage_b(0, sts[0]); stage_c(0, sts[0])
    elif nb == 2:
        stage_a(0, sts[0]); stage_a(1, sts[1])
        stage_b(0, sts[0]); stage_c(0, sts[0])
        stage_b(1, sts[1]); stage_c(1, sts[1])
    elif _os.environ.get("K_PIPE", "i") == "i":
        # op-granular interleave: fill MLP-chain stalls with bulk work
        _mark(nc, "A0")
        drive(_ga(0, sts[0]))
        _mark(nc, "A1B0")
        drive(_gb(0, sts[0]), _ga(1, sts[1]))
        _mark(nc, "B1C0A2")
        drive(_gb(1, sts[1]), _gc(0, sts[0]), _ga(2, sts[2]))
        _mark(nc, "B2C1A3")
        drive(_gb(2, sts[2]), _gc(1, sts[1]), _ga(3, sts[3]))
        _mark(nc, "B3C2")
        drive(_gb(3, sts[3]), _gc(2, sts[2]))
        _mark(nc, "C3")
        drive(_gc(3, sts[3]))
    elif _os.environ.get("K_PIPE", "i") == "3":
        stage_a(0, sts[0])
        stage_a(1, sts[1])
        stage_b(0, sts[0])
        stage_a(2, sts[2])
        stage_c(0, sts[0])
        stage_b(1, sts[1])
        stage_a(3, sts[3])
        stage_c(1, sts[1])
        stage_b(2, sts[2])
        stage_c(2, sts[2])
        stage_b(3, sts[3])
        stage_c(3, sts[3])
    else:
        stage_a(0, sts[0])
        stage_a(1, sts[1])
        stage_b(0, sts[0])
        stage_c(0, sts[0])
        stage_a(2, sts[2])
        stage_b(1, sts[1])
        stage_c(1, sts[1])
        stage_a(3, sts[3])
        stage_b(2, sts[2])
        stage_c(2, sts[2])
        stage_b(3, sts[3])
        stage_c(3, sts[3])

    ctx.close()


_ACT_TABLES_PATCHED = False


def _pin_act_table():
    """Force the act-table chooser onto natural_log_exp_and_others (which
    covers Copy/Ln/Exp/Relu) so the whole kernel needs ONE table load
    instead of thrashing between exp_and_others and natural_log."""
    global _ACT_TABLES_PATCHED
    if _ACT_TABLES_PATCHED:
        return
    from concourse import hw_specs
    import concourse.bacc as bacc_mod
    orig = hw_specs.get_activation_tables
    mine = {AF.Copy, AF.Ln, AF.Exp, AF.Relu, AF.Identity}
    keep = "natural_log_exp_and_others"

    def patched(arch):
        tabs = orig(arch)
        if keep not in tabs or not mine <= tabs[keep]:
            return tabs
        return {
            name: (s if name == keep else s - mine)
            for name, s in tabs.items()
        }

    bacc_mod.get_activation_tables = patched
    _ACT_TABLES_PATCHED = True


def build_program(inputs):
    """Build + compile the SPMD Bass program with folded params baked in."""
    _pin_act_table()
    w1, b1, W2, b2, wp2, bp2 = _fold_params(inputs)
    nc = bacc.Bacc(
        "TRN2",
        target_bir_lowering=False,
        debug=False,
        enable_asserts=False,
        num_devices=N_CORES,
    )
    x_d = nc.dram_tensor("x", (B_PER_CORE, T, C), F32, kind="ExternalInput").ap()
    o_d = nc.dram_tensor("out", (B_PER_CORE, T, C), F32, kind="ExternalOutput").ap()
    with tile.TileContext(nc) as tc:
        _build_kernel(tc, o_d, x_d, w1, b1, W2, b2, wp2, bp2)
    nc.compile()
    return nc


def run_device(nc, x, trace=False, **kw):
    """Run the compiled program over the 8 cores; return (out, results)."""
    x = np.asarray(x, np.float32)
    in_maps = [
        {"x": np.ascontiguousarray(x[c * B_PER_CORE:(c + 1) * B_PER_CORE])}
        for c in range(N_CORES)
    ]
    res = bass_utils.run_bass_kernel_spmd(
        nc, in_maps, core_ids=list(range(N_CORES)), trace=trace, **kw
    )
    out = np.concatenate(
        [res.results[c]["out"] for c in range(N_CORES)], axis=0
    )
    return out, res


def host_penalty(inputs):
    """Monotonicity penalty on 21 keypoints; float32 math mirroring reference."""
    f32 = np.float32
    W1 = np.asarray(inputs["W1"], f32)
    b1 = np.asarray(inputs["b1"], f32)
    W2 = np.asarray(inputs["W2"], f32)
    b2 = np.asarray(inputs["b2"], f32)
    gamma = np.asarray(inputs["gamma"], f32)
    beta = np.asarray(inputs["beta"], f32)
    mmean = np.asarray(inputs["mmean"], f32)
    mvar = np.asarray(inputs["mvar"], f32)
    Wp = np.asarray(inputs["Wp"], f32)
    bp = np.asarray(inputs["bp"], f32)

    def elu(v):
        return np.where(v > 0, v, np.expm1(v)).astype(f32)

    z = np.linspace(-1.0, 1.0, 21).astype(f32).reshape(-1, 1)
    h = elu(z @ W1 + b1)
    h = elu(h @ W2 + b2)
    h = (h - mmean) * (f32(1.0) / np.sqrt(mvar + f32(BN_EPS))) * gamma + beta
    o = h @ Wp + bp
    kout = (f32(1.0) / (f32(1.0) + np.exp(-o)))[:, 0]
    dL = kout[1:11] - kout[:10]
    dR = kout[11:] - kout[10:-1]
    pen = f32(0.5) * f32(PENALTY_RATE) * np.mean(
        np.abs(dL) - dL + np.abs(dR) - dR, dtype=f32
    )
    return f32(pen)


def kernel(**inputs):
    x = np.asarray(inputs["x"], np.float32)
    nc = build_program(inputs)
    out, _ = run_device(nc, x)
    penalty = host_penalty(inputs)
    return out, penalty


# revision 28
# speedup vs baseline: 33900.4506x; 1.0101x over previous
"""Trainium2 Bass kernel for nn_FC_mono_12086037971055 (dense_mlp).

Computation (per batch b of x: (T=4096, C=256)):
  norm_x = x / sqrt(sum_t x^2 + 1e-7)          (column-normalize over T)
  cor    = norm_x^T @ norm_x                   (C x C Gram of correlations)
  att    = MLP(cor) elementwise                (1->4 elu ->4 elu -> BN -> 1 sigmoid)
  att    = att / (sum_axis1(att) + 1e-7)       (column-normalize)
  out    = x + x @ (offdiag * att)
plus a scalar monotonicity penalty from 21 keypoints (computed host-side: it
depends only on the tiny MLP params).

Sharding: data-parallel over batch B=32 -> 4 batches on each of 8 cores.
All MLP/BN parameters are folded on the host into scalar immediates that are
baked into the Bass program (BN is affine at inference, so it folds into the
final dense layer).

Kernel strategy per batch (one NeuronCore):
  - G = X^T X with raw X via PE matmul in float32r (full rate at N=256).
    Column norms Q_c are G's diagonal, extracted with an eye-mask reduce.
  - cor = s_c * s_d * G with s = 1/sqrt(Q+eps).  The per-free-dim scale is
    applied via PE transposes of row-scaled blocks (G is symmetric, so only
    blocks 00, 10, 11 are needed; 10 comes from transposing row-scaled 01).
  - MLP runs elementwise on one packed (128, 384) tile.
    elu(u) = min(exp(u) - 1, relu(u)), exp/relu/sigmoid on ScalarE.
  - att column-normalize: row-sums (symmetry) -> R = 1/(S+eps); the
    per-column application is again row-scale + PE transpose.  The offdiag
    mask and the +Identity fold (out = X @ (offdiag*att + I)) are applied
    during the PSUM->SBUF copies.
  - out rows: lhsT = X^T (built with PE transposes of X), rhs = A''.
"""

import numpy as np

import concourse.bass as bass
import concourse.tile as tile
from concourse import bacc, bass_utils, mybir

F32 = mybir.dt.float32
F32R = mybir.dt.float32r
F16 = mybir.dt.float16
AF = mybir.ActivationFunctionType
ALU = mybir.AluOpType
X_AX = mybir.AxisListType.X

N_CORES = 8
B_TOTAL = 32
B_PER_CORE = B_TOTAL // N_CORES   # 4
T = 4096
C = 256
P = 128
NT = T // P                        # 32 t-blocks
NCHUNK = 8                         # x streamed in 8 chunks per batch
LT = NT // NCHUNK                  # 8 t-blocks per chunk
BN_EPS = 1e-3
EPS = 1e-7
PENALTY_RATE = 10.0


def _fold_params(inputs):
    """Fold BN into the projection layer; return plain python floats."""
    W1 = np.asarray(inputs["W1"], np.float64)     # (1, 4)
    b1 = np.asarray(inputs["b1"], np.float64)     # (4,)
    W2 = np.asarray(inputs["W2"], np.float64)     # (4, 4)
    b2 = np.asarray(inputs["b2"], np.float64)     # (4,)
    gamma = np.asarray(inputs["gamma"], np.float64)
    beta = np.asarray(inputs["beta"], np.float64)
    mmean = np.asarray(inputs["mmean"], np.float64)
    mvar = np.asarray(inputs["mvar"], np.float64)
    Wp = np.asarray(inputs["Wp"], np.float64)     # (4, 1)
    bp = np.asarray(inputs["bp"], np.float64)     # (1,)

    a = gamma / np.sqrt(mvar + BN_EPS)
    wp2 = Wp[:, 0] * a
    bp2 = bp[0] + np.sum(Wp[:, 0] * (beta - mmean * a))
    return (
        [float(v) for v in W1[0]],
        [float(v) for v in b1],
        [[float(W2[i, j]) for j in range(4)] for i in range(4)],
        [float(v) for v in b2],
        [float(v) for v in wp2],
        float(bp2),
    )


class _St:
    pass


STAGE_RANGES = []


def _mark(nc, label):
    STAGE_RANGES.append((label, int(nc.next_id())))


def _build_kernel(tc, o_d, x_d, w1, b1, W2, b2, wp2, bp2):
    nc = tc.nc

    import contextlib
    ctx = contextlib.ExitStack()
    consts = ctx.enter_context(tc.tile_pool(name="consts", bufs=1))
    xin = ctx.enter_context(tc.tile_pool(name="xin", bufs=6))
    xrp = ctx.enter_context(tc.tile_pool(name="xrp", bufs=6))
    xtp = ctx.enter_context(tc.tile_pool(name="xtp", bufs=6))
    mlp = ctx.enter_context(tc.tile_pool(name="mlp", bufs=2))
    small = ctx.enter_context(tc.tile_pool(name="small", bufs=2))
    apool = ctx.enter_context(tc.tile_pool(name="apool", bufs=2))
    ostage = ctx.enter_context(tc.tile_pool(name="ostage", bufs=4))
    pp_gram = ctx.enter_context(tc.tile_pool(name="pp_gram", bufs=1, space="PSUM"))
    pp_xt = ctx.enter_context(tc.tile_pool(name="pp_xt", bufs=2, space="PSUM"))
    pp_mlp = ctx.enter_context(tc.tile_pool(name="pp_mlp", bufs=2, space="PSUM"))
    pp_out = ctx.enter_context(tc.tile_pool(name="pp_out", bufs=2, space="PSUM"))

    # constants: identity (also the eye mask) and offdiag = 1 - eye
    from concourse.masks import make_identity
    ident = consts.tile([P, P], F32, tag="ident")
    make_identity(nc, ident)
    identr = consts.tile([P, P], F32R, tag="identr")
    nc.vector.tensor_copy(identr, ident)
    od = consts.tile([P, P], F32, tag="od")
    nc.vector.tensor_scalar(
        out=od, in0=ident, scalar1=-1.0, scalar2=1.0, op0=ALU.mult, op1=ALU.add
    )

    # (128,1) constant tiles used as activation biases
    bias_tiles = {}

    def cbias(val):
        v = float(val)
        if v not in bias_tiles:
            t = consts.tile(
                [P, 1], F32, tag=f"cb{len(bias_tiles)}",
                name=f"cb{len(bias_tiles)}",
            )
            nc.vector.memset(t, v)
            bias_tiles[v] = t
        return bias_tiles[v]

    def r32(ap):
        return ap.bitcast(F32R)

    # PE warmup: keep TensorE busy through the initial DMA fill so the
    # first real Gram matmuls run at full clock (p-state ramp done).
    warm = pp_mlp.tile([P, P], F32, tag="mlpT", name="warm")
    for _ in range(36):
        nc.tensor.matmul(warm, identr, identr, start=True, stop=True)

    # ---------------- stage A: load, Gram, X^T, cor -> m_in ----------------
    def stage_a(b, st):
        # load x[b] in 4 chunks of (1024, 256)
        st.X = []
        for g in range(NCHUNK):
            xg = xin.tile([P, LT, C], F32, tag="xin")
            src = x_d[b, g * LT * P:(g + 1) * LT * P, :].rearrange(
                "(lt p) c -> p lt c", p=P
            )
            nc.sync.dma_start(out=xg, in_=src)
            # round to f32r for full-rate PE consumption (GPSIMD is idle)
            xr = xrp.tile([P, LT, C], F32R, tag="xr")
            nc.gpsimd.tensor_copy(xr, xg)
            st.X.append(xr)

        st.XT = [
            xtp.tile([P, T], F32R, tag="xt", name=f"xt_b{b}_c{cb}")
            for cb in range(2)
        ]

        G0 = pp_gram.tile([P, C], F32, tag="g0")
        G1 = pp_gram.tile([P, C], F32, tag="g1")

        for g in range(NCHUNK):
            yield
            xg = st.X[g]
            for lt in range(LT):
                k = g * LT + lt
                xa = xg[:, lt, :]
                nc.tensor.matmul(
                    G0[:, :], xa[:, 0:P], xa,
                    start=(k == 0), stop=(k == NT - 1),
                )
                nc.tensor.matmul(
                    G1[:, :], xa[:, P:C], xa,
                    start=(k == 0), stop=(k == NT - 1),
                )
            # transposes of this chunk into X^T
            for cb in range(2):
                for h in range(LT // 4):
                    tg = pp_xt.tile([P, 4 * P], F32R, tag="xtT")
                    for i in range(4):
                        lt = h * 4 + i
                        nc.tensor.transpose(
                            tg[:, i * P:(i + 1) * P],
                            xg[:, lt, cb * P:(cb + 1) * P],
                            identr,
                        )
                    t0 = (g * LT + h * 4) * P
                    dst = st.XT[cb][:, t0:t0 + 4 * P]
                    if (g * 2 + cb + h) % 4 != 3:
                        nc.scalar.copy(dst, tg)
                    else:
                        nc.vector.tensor_copy(dst, tg)

        # copy G out of PSUM immediately so the next batch's Gram can start
        Gs0 = small.tile([P, C], F32, tag="gs0")
        Gs1 = small.tile([P, P], F32, tag="gs1")
        nc.scalar.copy(Gs0, G0)
        nc.scalar.copy(Gs1, G1[:, P:C])

        # column norms from the Gram diagonal
        scr = small.tile([P, P], F32, tag="scr")
        Q0 = small.tile([P, 1], F32, tag="q0")
        Q1 = small.tile([P, 1], F32, tag="q1")
        nc.vector.tensor_mul(scr, Gs0[:, 0:P], ident)
        nc.vector.tensor_reduce(Q0, scr, axis=X_AX, op=ALU.add)
        scr2 = small.tile([P, P], F32, tag="scr2")
        nc.vector.tensor_mul(scr2, Gs1, ident)
        nc.vector.tensor_reduce(Q1, scr2, axis=X_AX, op=ALU.add)
        sq0 = small.tile([P, 1], F32, tag="sq0")
        sq1 = small.tile([P, 1], F32, tag="sq1")
        nc.scalar.activation(sq0, Q0, AF.Ln, bias=cbias(EPS), scale=1.0)
        nc.scalar.activation(sq1, Q1, AF.Ln, bias=cbias(EPS), scale=1.0)
        s0 = small.tile([P, 1], F32, tag="s0")
        s1 = small.tile([P, 1], F32, tag="s1")
        nc.scalar.activation(s0, sq0, AF.Exp, bias=cbias(0.0), scale=-0.5)
        nc.scalar.activation(s1, sq1, AF.Exp, bias=cbias(0.0), scale=-0.5)

        # cor blocks: row-scale, transpose, scale again on the copy out
        u00 = small.tile([P, P], F32, tag="u00")
        u01 = small.tile([P, P], F32, tag="u01")
        u11 = small.tile([P, P], F32, tag="u11")
        nc.vector.tensor_scalar_mul(u00, Gs0[:, 0:P], s0)
        nc.vector.tensor_scalar_mul(u01, Gs0[:, P:C], s0)
        nc.vector.tensor_scalar_mul(u11, Gs1, s1)
        tgc = pp_mlp.tile([P, 3 * P], F32, tag="mlpT")
        nc.tensor.transpose(tgc[:, 0:P], u00, ident)
        nc.tensor.transpose(tgc[:, P:2 * P], u01, ident)
        nc.tensor.transpose(tgc[:, 2 * P:3 * P], u11, ident)
        m_in = mlp.tile([P, 3 * P], F32, tag="m_in", bufs=3)
        nc.vector.tensor_scalar_mul(m_in[:, 0:P], tgc[:, 0:P], s0)
        nc.vector.tensor_scalar_mul(m_in[:, P:2 * P], tgc[:, P:2 * P], s1)
        nc.vector.tensor_scalar_mul(m_in[:, 2 * P:3 * P], tgc[:, 2 * P:3 * P], s1)
        st.m_in = m_in

    # ---------------- stage B: MLP + normalize -> A'' ----------------
    def stage_b(b, st):
        m_in = st.m_in
        W = 3 * P

        def elu_pair(src, scale, bias):
            E = mlp.tile([P, W], F32, tag="e")
            R = mlp.tile([P, W], F32, tag="r")
            nc.scalar.activation(E, src, AF.Exp, bias=bias, scale=scale)
            nc.scalar.activation(R, src, AF.Relu, bias=bias, scale=scale)
            h = mlp.tile([P, W], F32, tag="h")
            nc.vector.scalar_tensor_tensor(
                h, in0=E, scalar=-1.0, in1=R, op0=ALU.add, op1=ALU.min
            )
            return h

        H1 = []
        for i in range(4):
            E = mlp.tile([P, W], F16, tag="e")
            R = mlp.tile([P, W], F16, tag="r")
            nc.scalar.activation(E, m_in, AF.Exp, bias=cbias(b1[i]), scale=w1[i])
            import os as _os
            if b1[i] == 0.0 and _os.environ.get("K_R1", "act") == "dve":
                nc.vector.tensor_scalar(
                    out=R, in0=m_in, scalar1=w1[i], scalar2=0.0,
                    op0=ALU.mult, op1=ALU.max,
                )
            else:
                nc.scalar.activation(R, m_in, AF.Relu, bias=cbias(b1[i]), scale=w1[i])
            h = mlp.tile([P, W], F16, tag=f"h1_{i}", bufs=1)
            nc.vector.scalar_tensor_tensor(
                h, in0=E, scalar=-1.0, in1=R, op0=ALU.add, op1=ALU.min
            )
            H1.append(h)
            if i % 2 == 1:
                yield

        H2 = []
        for j in range(4):
            import os as _os
            _l2g = _os.environ.get("K_L2G", "dve")
            if _l2g == "odd":
                eng = nc.gpsimd if j % 2 == 1 else nc.vector
            elif _l2g == "j3":
                eng = nc.gpsimd if j == 3 else nc.vector
            else:
                eng = nc.vector
            cur = mlp.tile([P, W], F16, tag="u")
            eng.tensor_scalar(
                out=cur, in0=H1[0], scalar1=W2[0][j], scalar2=b2[j],
                op0=ALU.mult, op1=ALU.add,
            )
            for i in range(1, 4):
                nxt = mlp.tile([P, W], F16, tag="u")
                eng.scalar_tensor_tensor(
                    nxt, in0=H1[i], scalar=W2[i][j], in1=cur,
                    op0=ALU.mult, op1=ALU.add,
                )
                cur = nxt
            E = mlp.tile([P, W], F16, tag="e")
            R = mlp.tile([P, W], F16, tag="r")
            nc.scalar.activation(E, cur, AF.Exp, bias=cbias(0.0))
            nc.scalar.activation(R, cur, AF.Relu, bias=cbias(0.0))
            h = mlp.tile([P, W], F16, tag=f"h2_{j}", bufs=1)
            nc.vector.scalar_tensor_tensor(
                h, in0=E, scalar=-1.0, in1=R, op0=ALU.add, op1=ALU.min
            )
            H2.append(h)
            yield

        import os as _os
        _l3 = nc.gpsimd if _os.environ.get("K_L3", "dve") == "gps" else nc.vector
        cur = mlp.tile([P, W], F16, tag="u")
        _l3.tensor_scalar(
            out=cur, in0=H2[0], scalar1=wp2[0], scalar2=bp2,
            op0=ALU.mult, op1=ALU.add,
        )
        for j in range(1, 4):
            nxt = mlp.tile([P, W], F16, tag="u")
            _l3.scalar_tensor_tensor(
                nxt, in0=H2[j], scalar=wp2[j], in1=cur, op0=ALU.mult, op1=ALU.add
            )
            cur = nxt
        eneg = mlp.tile([P, W], F16, tag="e")
        nc.scalar.activation(eneg, cur, AF.Exp, bias=cbias(0.0), scale=-1.0)
        wden = mlp.tile([P, W], F32, tag="u")
        nc.vector.tensor_scalar_add(wden, eneg, 1.0)
        a_out = mlp.tile([P, W], F32, tag="a_out")
        nc.vector.reciprocal(a_out, wden)
        yield

        # A01 = A10^T  (att pre-normalization is symmetric)
        t01p = pp_mlp.tile([P, 3 * P], F32, tag="mlpT")
        nc.tensor.transpose(t01p[:, 0:P], a_out[:, P:2 * P], ident)
        t01 = small.tile([P, P], F32, tag="t01")
        S01 = small.tile([P, 1], F32, tag="s01")
        nc.scalar.copy(t01, t01p[:, 0:P])
        nc.vector.tensor_reduce(S01, t01, axis=X_AX, op=ALU.add)

        # column sums via row sums (symmetry)
        Sa = small.tile([P, 1], F32, tag="sa")
        nc.vector.tensor_reduce(Sa, a_out[:, 0:P], axis=X_AX, op=ALU.add)
        Sc0 = small.tile([P, 1], F32, tag="sc0")
        nc.vector.tensor_add(Sc0, Sa, S01)
        Sc1 = small.tile([P, 1], F32, tag="sc1")
        nc.vector.tensor_reduce(Sc1, a_out[:, P:3 * P], axis=X_AX, op=ALU.add)
        yield
        R0 = small.tile([P, 1], F32, tag="r0")
        R1 = small.tile([P, 1], F32, tag="r1")
        t0 = small.tile([P, 1], F32, tag="t0")
        t1 = small.tile([P, 1], F32, tag="t1")
        nc.vector.tensor_scalar_add(t0, Sc0, EPS)
        nc.vector.tensor_scalar_add(t1, Sc1, EPS)
        nc.vector.reciprocal(R0, t0)
        nc.vector.reciprocal(R1, t1)

        # U = R * (A * offdiag-mask), blockwise
        n00 = small.tile([P, P], F32, tag="n00")
        n11 = small.tile([P, P], F32, tag="n11")
        nc.vector.tensor_mul(n00, a_out[:, 0:P], od)
        nc.vector.tensor_mul(n11, a_out[:, 2 * P:3 * P], od)
        v00 = small.tile([P, P], F32, tag="v00")
        v01 = small.tile([P, P], F32, tag="v01")
        v10 = small.tile([P, P], F32, tag="v10")
        v11 = small.tile([P, P], F32, tag="v11")
        nc.vector.tensor_scalar_mul(v00, n00, R0)
        nc.vector.tensor_scalar_mul(v01, t01, R0)
        nc.vector.tensor_scalar_mul(v10, a_out[:, P:2 * P], R1)
        nc.vector.tensor_scalar_mul(v11, n11, R1)

        # A'' = U^T + I
        p0 = pp_mlp.tile([P, 3 * P], F32, tag="mlpT")
        nc.tensor.transpose(p0[:, 0:P], v00, ident)
        nc.tensor.transpose(p0[:, P:2 * P], v10, ident)
        p1 = pp_mlp.tile([P, 3 * P], F32, tag="mlpT")
        nc.tensor.transpose(p1[:, 0:P], v01, ident)
        nc.tensor.transpose(p1[:, P:2 * P], v11, ident)
        A0 = apool.tile([P, C], F32R, tag="A0")
        A1 = apool.tile([P, C], F32R, tag="A1")
        nc.vector.scalar_tensor_tensor(
            A0[:, 0:P], in0=p0[:, 0:P], scalar=1.0, in1=ident,
            op0=ALU.mult, op1=ALU.add,
        )
        nc.scalar.copy(A0[:, P:C], p0[:, P:2 * P])
        nc.scalar.copy(A1[:, 0:P], p1[:, 0:P])
        nc.vector.scalar_tensor_tensor(
            A1[:, P:C], in0=p1[:, P:2 * P], scalar=1.0, in1=ident,
            op0=ALU.mult, op1=ALU.add,
        )
        st.A = (A0, A1)

    # ---------------- stage C: out = X @ A'' ----------------
    def stage_c(b, st):
        A0, A1 = st.A
        XT0, XT1 = st.XT
        for g in range(NCHUNK):
            ost = ostage.tile([P, LT, C], F32, tag="ost")
            for lt in range(LT):
                yield
                tb = g * LT + lt
                po = pp_out.tile([P, C], F32, tag="po")
                nc.tensor.matmul(
                    po, XT0[:, tb * P:(tb + 1) * P], A0,
                    start=True, stop=False,
                )
                nc.tensor.matmul(
                    po, XT1[:, tb * P:(tb + 1) * P], A1,
                    start=False, stop=True,
                )
                if lt % 2 == 0:
                    nc.scalar.copy(ost[:, lt, :], po)
                else:
                    nc.vector.tensor_copy(ost[:, lt, :], po)
            dst = o_d[b, g * LT * P:(g + 1) * LT * P, :].rearrange(
                "(lt p) c -> p lt c", p=P
            )
            nc.sync.dma_start(out=dst, in_=ost)

    # ---------------- software-pipelined emission ----------------
    import os as _os
    nb = int(_os.environ.get("KERNEL_NBATCH", str(B_PER_CORE)))
    _ = _os
    sts = [_St() for _ in range(B_PER_CORE)]
    del STAGE_RANGES[:]
    _ga, _gb, _gc = stage_a, stage_b, stage_c

    def drive(*gens):
        live = list(gens)
        while live:
            for g in list(live):
                try:
                    next(g)
                except StopIteration:
                    live.remove(g)

    def _tag(label, gen):
        _mark(nc, label)
        return gen

    def stage_a(b, st):
        _mark(nc, f"A{b}")
        drive(_ga(b, st))

    def stage_b(b, st):
        _mark(nc, f"B{b}")
        drive(_gb(b, st))

    def stage_c(b, st):
        _mark(nc, f"C{b}")
        drive(_gc(b, st))

    if nb == 1:
        stage_a(0, sts[0]); stage_b(0, sts[0]); stage_c(0, sts[0])
    elif nb == 2:
        stage_a(0, sts[0]); stage_a(1, sts[1])
        stage_b(0, sts[0]); stage_c(0, sts[0])
        stage_b(1, sts[1]); stage_c(1, sts[1])
    elif _os.environ.get("K_PIPE", "i") == "i":
        # op-granular interleave: fill MLP-chain stalls with bulk work
        _mark(nc, "A0")
        drive(_ga(0, sts[0]))
        _mark(nc, "A1B0")
        drive(_gb(0, sts[0]), _ga(1, sts[1]))
        _mark(nc, "B1C0A2")
        drive(_gb(1, sts[1]), _gc(0, sts[0]), _ga(2, sts[2]))
        _mark(nc, "B2C1A3")
        drive(_gb(2, sts[2]), _gc(1, sts[1]), _ga(3, sts[3]))
        _mark(nc, "B3C2")
        drive(_gb(3, sts[3]), _gc(2, sts[2]))
        _mark(nc, "C3")
        drive(_gc(3, sts[3]))
    elif _os.environ.get("K_PIPE", "i") == "3":
        stage_a(0, sts[0])
        stage_a(1, sts[1])
        stage_b(0, sts[0])
        stage_a(2, sts[2])
        stage_c(0, sts[0])
        stage_b(1, sts[1])
        stage_a(3, sts[3])
        stage_c(1, sts[1])
        stage_b(2, sts[2])
        stage_c(2, sts[2])
        stage_b(3, sts[3])
        stage_c(3, sts[3])
    else:
        stage_a(0, sts[0])
        stage_a(1, sts[1])
        stage_b(0, sts[0])
        stage_c(0, sts[0])
        stage_a(2, sts[2])
        stage_b(1, sts[1])
        stage_c(1, sts[1])
        stage_a(3, sts[3])
        stage_b(2, sts[2])
        stage_c(2, sts[2])
        stage_b(3, sts[3])
        stage_c(3, sts[3])

    ctx.close()


_ACT_TABLES_PATCHED = False


def _pin_act_table():
    """Force the act-table chooser onto natural_log_exp_and_others (which
    covers Copy/Ln/Exp/Relu) so the whole kernel needs ONE table load
    instead of thrashing between exp_and_others and natural_log."""
    global _ACT_TABLES_PATCHED
    if _ACT_TABLES_PATCHED:
        return
    from concourse import hw_specs
    import concourse.bacc as bacc_mod
    orig = hw_specs.get_activation_tables
    mine = {AF.Copy, AF.Ln, AF.Exp, AF.Relu, AF.Identity}
    keep = "natural_log_exp_and_others"

    def patched(arch):
        tabs = orig(arch)
        if keep not in tabs or not mine <= tabs[keep]:
            return tabs
        return {
            name: (s if name == keep else s - mine)
            for name, s in tabs.items()
        }

    bacc_mod.get_activation_tables = patched
    _ACT_TABLES_PATCHED = True


def build_program(inputs):
    """Build + compile the SPMD Bass program with folded params baked in."""
    _pin_act_table()
    w1, b1, W2, b2, wp2, bp2 = _fold_params(inputs)
    nc = bacc.Bacc(
        "TRN2",
        target_bir_lowering=False,
        debug=False,
        enable_asserts=False,
        num_devices=N_CORES,
    )
    x_d = nc.dram_tensor("x", (B_PER_CORE, T, C), F32, kind="ExternalInput").ap()
    o_d = nc.dram_tensor("out", (B_PER_CORE, T, C), F32, kind="ExternalOutput").ap()
    with tile.TileContext(nc) as tc:
        _build_kernel(tc, o_d, x_d, w1, b1, W2, b2, wp2, bp2)
    nc.compile()
    return nc


def run_device(nc, x, trace=False, **kw):
    """Run the compiled program over the 8 cores; return (out, results)."""
    x = np.asarray(x, np.float32)
    in_maps = [
        {"x": np.ascontiguousarray(x[c * B_PER_CORE:(c + 1) * B_PER_CORE])}
        for c in range(N_CORES)
    ]
    res = bass_utils.run_bass_kernel_spmd(
        nc, in_maps, core_ids=list(range(N_CORES)), trace=trace, **kw
    )
    out = np.concatenate(
        [res.results[c]["out"] for c in range(N_CORES)], axis=0
    )
    return out, res


def host_penalty(inputs):
    """Monotonicity penalty on 21 keypoints; float32 math mirroring reference."""
    f32 = np.float32
    W1 = np.asarray(inputs["W1"], f32)
    b1 = np.asarray(inputs["b1"], f32)
    W2 = np.asarray(inputs["W2"], f32)
    b2 = np.asarray(inputs["b2"], f32)
    gamma = np.asarray(inputs["gamma"], f32)
    beta = np.asarray(inputs["beta"], f32)
    mmean = np.asarray(inputs["mmean"], f32)
    mvar = np.asarray(inputs["mvar"], f32)
    Wp = np.asarray(inputs["Wp"], f32)
    bp = np.asarray(inputs["bp"], f32)

    def elu(v):
        return np.where(v > 0, v, np.expm1(v)).astype(f32)

    z = np.linspace(-1.0, 1.0, 21).astype(f32).reshape(-1, 1)
    h = elu(z @ W1 + b1)
    h = elu(h @ W2 + b2)
    h = (h - mmean) * (f32(1.0) / np.sqrt(mvar + f32(BN_EPS))) * gamma + beta
    o = h @ Wp + bp
    kout = (f32(1.0) / (f32(1.0) + np.exp(-o)))[:, 0]
    dL = kout[1:11] - kout[:10]
    dR = kout[11:] - kout[10:-1]
    pen = f32(0.5) * f32(PENALTY_RATE) * np.mean(
        np.abs(dL) - dL + np.abs(dR) - dR, dtype=f32
    )
    return f32(pen)


def kernel(**inputs):
    x = np.asarray(inputs["x"], np.float32)
    nc = build_program(inputs)
    out, _ = run_device(nc, x)
    penalty = host_penalty(inputs)
    return out, penalty


# revision 29
# speedup vs baseline: 36510.2819x; 1.0770x over previous
"""Trainium2 Bass kernel for nn_FC_mono_12086037971055 (dense_mlp).

Computation (per batch b of x: (T=4096, C=256)):
  norm_x = x / sqrt(sum_t x^2 + 1e-7)          (column-normalize over T)
  cor    = norm_x^T @ norm_x                   (C x C Gram of correlations)
  att    = MLP(cor) elementwise                (1->4 elu ->4 elu -> BN -> 1 sigmoid)
  att    = att / (sum_axis1(att) + 1e-7)       (column-normalize)
  out    = x + x @ (offdiag * att)
plus a scalar monotonicity penalty from 21 keypoints (computed host-side: it
depends only on the tiny MLP params).

Sharding: data-parallel over batch B=32 -> 4 batches on each of 8 cores.
All MLP/BN parameters are folded on the host into scalar immediates that are
baked into the Bass program (BN is affine at inference, so it folds into the
final dense layer).

Kernel strategy per batch (one NeuronCore):
  - G = X^T X with raw X via PE matmul in float32r (full rate at N=256).
    Column norms Q_c are G's diagonal, extracted with an eye-mask reduce.
  - cor = s_c * s_d * G with s = 1/sqrt(Q+eps).  The per-free-dim scale is
    applied via PE transposes of row-scaled blocks (G is symmetric, so only
    blocks 00, 10, 11 are needed; 10 comes from transposing row-scaled 01).
  - MLP runs elementwise on one packed (128, 384) tile.
    elu(u) = min(exp(u) - 1, relu(u)), exp/relu/sigmoid on ScalarE.
  - att column-normalize: row-sums (symmetry) -> R = 1/(S+eps); the
    per-column application is again row-scale + PE transpose.  The offdiag
    mask and the +Identity fold (out = X @ (offdiag*att + I)) are applied
    during the PSUM->SBUF copies.
  - out rows: lhsT = X^T (built with PE transposes of X), rhs = A''.
"""

import numpy as np

import concourse.bass as bass
import concourse.tile as tile
from concourse import bacc, bass_utils, mybir

F32 = mybir.dt.float32
F32R = mybir.dt.float32r
F16 = mybir.dt.float16
AF = mybir.ActivationFunctionType
ALU = mybir.AluOpType
X_AX = mybir.AxisListType.X

N_CORES = 8
B_TOTAL = 32
B_PER_CORE = B_TOTAL // N_CORES   # 4
T = 4096
C = 256
P = 128
NT = T // P                        # 32 t-blocks
NCHUNK = 8                         # x streamed in 8 chunks per batch
LT = NT // NCHUNK                  # 8 t-blocks per chunk
BN_EPS = 1e-3
EPS = 1e-7
PENALTY_RATE = 10.0


def _fold_params(inputs):
    """Fold BN into the projection layer; return plain python floats."""
    W1 = np.asarray(inputs["W1"], np.float64)     # (1, 4)
    b1 = np.asarray(inputs["b1"], np.float64)     # (4,)
    W2 = np.asarray(inputs["W2"], np.float64)     # (4, 4)
    b2 = np.asarray(inputs["b2"], np.float64)     # (4,)
    gamma = np.asarray(inputs["gamma"], np.float64)
    beta = np.asarray(inputs["beta"], np.float64)
    mmean = np.asarray(inputs["mmean"], np.float64)
    mvar = np.asarray(inputs["mvar"], np.float64)
    Wp = np.asarray(inputs["Wp"], np.float64)     # (4, 1)
    bp = np.asarray(inputs["bp"], np.float64)     # (1,)

    a = gamma / np.sqrt(mvar + BN_EPS)
    wp2 = Wp[:, 0] * a
    bp2 = bp[0] + np.sum(Wp[:, 0] * (beta - mmean * a))
    return (
        [float(v) for v in W1[0]],
        [float(v) for v in b1],
        [[float(W2[i, j]) for j in range(4)] for i in range(4)],
        [float(v) for v in b2],
        [float(v) for v in wp2],
        float(bp2),
    )


class _St:
    pass


STAGE_RANGES = []


def _mark(nc, label):
    STAGE_RANGES.append((label, int(nc.next_id())))


def _build_kernel(tc, o_d, x_d, w1, b1, W2, b2, wp2, bp2):
    nc = tc.nc

    import contextlib
    ctx = contextlib.ExitStack()
    consts = ctx.enter_context(tc.tile_pool(name="consts", bufs=1))
    xin = ctx.enter_context(tc.tile_pool(name="xin", bufs=6))
    xrp = ctx.enter_context(tc.tile_pool(name="xrp", bufs=6))
    xtp = ctx.enter_context(tc.tile_pool(name="xtp", bufs=6))
    mlp = ctx.enter_context(tc.tile_pool(name="mlp", bufs=3))
    small = ctx.enter_context(tc.tile_pool(name="small", bufs=2))
    apool = ctx.enter_context(tc.tile_pool(name="apool", bufs=2))
    ostage = ctx.enter_context(tc.tile_pool(name="ostage", bufs=4))
    pp_gram = ctx.enter_context(tc.tile_pool(name="pp_gram", bufs=1, space="PSUM"))
    pp_xt = ctx.enter_context(tc.tile_pool(name="pp_xt", bufs=2, space="PSUM"))
    pp_mlp = ctx.enter_context(tc.tile_pool(name="pp_mlp", bufs=2, space="PSUM"))
    pp_out = ctx.enter_context(tc.tile_pool(name="pp_out", bufs=2, space="PSUM"))

    # constants: identity (also the eye mask) and offdiag = 1 - eye
    from concourse.masks import make_identity
    ident = consts.tile([P, P], F32, tag="ident")
    make_identity(nc, ident)
    identr = consts.tile([P, P], F32R, tag="identr")
    nc.vector.tensor_copy(identr, ident)
    od = consts.tile([P, P], F32, tag="od")
    nc.vector.tensor_scalar(
        out=od, in0=ident, scalar1=-1.0, scalar2=1.0, op0=ALU.mult, op1=ALU.add
    )

    # (128,1) constant tiles used as activation biases
    bias_tiles = {}

    def cbias(val):
        v = float(val)
        if v not in bias_tiles:
            t = consts.tile(
                [P, 1], F32, tag=f"cb{len(bias_tiles)}",
                name=f"cb{len(bias_tiles)}",
            )
            nc.vector.memset(t, v)
            bias_tiles[v] = t
        return bias_tiles[v]

    def r32(ap):
        return ap.bitcast(F32R)

    # PE warmup: keep TensorE busy through the initial DMA fill so the
    # first real Gram matmuls run at full clock (p-state ramp done).
    warm = pp_mlp.tile([P, P], F32, tag="mlpT", name="warm")
    for _ in range(36):
        nc.tensor.matmul(warm, identr, identr, start=True, stop=True)

    # ---------------- stage A: load, Gram, X^T, cor -> m_in ----------------
    def stage_a(b, st):
        # load x[b] in 4 chunks of (1024, 256)
        st.X = []
        for g in range(NCHUNK):
            xg = xin.tile([P, LT, C], F32, tag="xin")
            src = x_d[b, g * LT * P:(g + 1) * LT * P, :].rearrange(
                "(lt p) c -> p lt c", p=P
            )
            nc.sync.dma_start(out=xg, in_=src)
            # round to f32r for full-rate PE consumption (GPSIMD is idle)
            xr = xrp.tile([P, LT, C], F32R, tag="xr")
            nc.gpsimd.tensor_copy(xr, xg)
            st.X.append(xr)

        st.XT = [
            xtp.tile([P, T], F32R, tag="xt", name=f"xt_b{b}_c{cb}")
            for cb in range(2)
        ]

        G0 = pp_gram.tile([P, C], F32, tag="g0")
        G1 = pp_gram.tile([P, C], F32, tag="g1")

        for g in range(NCHUNK):
            yield
            xg = st.X[g]
            for lt in range(LT):
                k = g * LT + lt
                xa = xg[:, lt, :]
                nc.tensor.matmul(
                    G0[:, :], xa[:, 0:P], xa,
                    start=(k == 0), stop=(k == NT - 1),
                )
                nc.tensor.matmul(
                    G1[:, :], xa[:, P:C], xa,
                    start=(k == 0), stop=(k == NT - 1),
                )
            # transposes of this chunk into X^T
            for cb in range(2):
                for h in range(LT // 4):
                    tg = pp_xt.tile([P, 4 * P], F32R, tag="xtT")
                    for i in range(4):
                        lt = h * 4 + i
                        nc.tensor.transpose(
                            tg[:, i * P:(i + 1) * P],
                            xg[:, lt, cb * P:(cb + 1) * P],
                            identr,
                        )
                    t0 = (g * LT + h * 4) * P
                    dst = st.XT[cb][:, t0:t0 + 4 * P]
                    if (g * 2 + cb + h) % 4 != 3:
                        nc.scalar.copy(dst, tg)
                    else:
                        nc.vector.tensor_copy(dst, tg)

        # copy G out of PSUM immediately so the next batch's Gram can start
        Gs0 = small.tile([P, C], F32, tag="gs0")
        Gs1 = small.tile([P, P], F32, tag="gs1")
        nc.scalar.copy(Gs0, G0)
        nc.scalar.copy(Gs1, G1[:, P:C])

        # column norms from the Gram diagonal
        scr = small.tile([P, P], F32, tag="scr")
        Q0 = small.tile([P, 1], F32, tag="q0")
        Q1 = small.tile([P, 1], F32, tag="q1")
        nc.vector.tensor_mul(scr, Gs0[:, 0:P], ident)
        nc.vector.tensor_reduce(Q0, scr, axis=X_AX, op=ALU.add)
        scr2 = small.tile([P, P], F32, tag="scr2")
        nc.vector.tensor_mul(scr2, Gs1, ident)
        nc.vector.tensor_reduce(Q1, scr2, axis=X_AX, op=ALU.add)
        sq0 = small.tile([P, 1], F32, tag="sq0")
        sq1 = small.tile([P, 1], F32, tag="sq1")
        nc.scalar.activation(sq0, Q0, AF.Ln, bias=cbias(EPS), scale=1.0)
        nc.scalar.activation(sq1, Q1, AF.Ln, bias=cbias(EPS), scale=1.0)
        s0 = small.tile([P, 1], F32, tag="s0")
        s1 = small.tile([P, 1], F32, tag="s1")
        nc.scalar.activation(s0, sq0, AF.Exp, bias=cbias(0.0), scale=-0.5)
        nc.scalar.activation(s1, sq1, AF.Exp, bias=cbias(0.0), scale=-0.5)

        # cor blocks: row-scale, transpose, scale again on the copy out
        u00 = small.tile([P, P], F32, tag="u00")
        u01 = small.tile([P, P], F32, tag="u01")
        u11 = small.tile([P, P], F32, tag="u11")
        nc.vector.tensor_scalar_mul(u00, Gs0[:, 0:P], s0)
        nc.vector.tensor_scalar_mul(u01, Gs0[:, P:C], s0)
        nc.vector.tensor_scalar_mul(u11, Gs1, s1)
        tgc = pp_mlp.tile([P, 3 * P], F32, tag="mlpT")
        nc.tensor.transpose(tgc[:, 0:P], u00, ident)
        nc.tensor.transpose(tgc[:, P:2 * P], u01, ident)
        nc.tensor.transpose(tgc[:, 2 * P:3 * P], u11, ident)
        m_in = mlp.tile([P, 3 * P], F32, tag="m_in", bufs=3)
        nc.vector.tensor_scalar_mul(m_in[:, 0:P], tgc[:, 0:P], s0)
        nc.vector.tensor_scalar_mul(m_in[:, P:2 * P], tgc[:, P:2 * P], s1)
        nc.vector.tensor_scalar_mul(m_in[:, 2 * P:3 * P], tgc[:, 2 * P:3 * P], s1)
        st.m_in = m_in

    # ---------------- stage B: MLP + normalize -> A'' ----------------
    def stage_b(b, st):
        m_in = st.m_in
        W = 3 * P

        def elu_pair(src, scale, bias):
            E = mlp.tile([P, W], F32, tag="e")
            R = mlp.tile([P, W], F32, tag="r")
            nc.scalar.activation(E, src, AF.Exp, bias=bias, scale=scale)
            nc.scalar.activation(R, src, AF.Relu, bias=bias, scale=scale)
            h = mlp.tile([P, W], F32, tag="h")
            nc.vector.scalar_tensor_tensor(
                h, in0=E, scalar=-1.0, in1=R, op0=ALU.add, op1=ALU.min
            )
            return h

        H1 = []
        for i in range(4):
            E = mlp.tile([P, W], F16, tag="e")
            R = mlp.tile([P, W], F16, tag="r")
            nc.scalar.activation(E, m_in, AF.Exp, bias=cbias(b1[i]), scale=w1[i])
            import os as _os
            if b1[i] == 0.0 and _os.environ.get("K_R1", "act") == "dve":
                nc.vector.tensor_scalar(
                    out=R, in0=m_in, scalar1=w1[i], scalar2=0.0,
                    op0=ALU.mult, op1=ALU.max,
                )
            else:
                nc.scalar.activation(R, m_in, AF.Relu, bias=cbias(b1[i]), scale=w1[i])
            h = mlp.tile([P, W], F16, tag=f"h1_{i}", bufs=1)
            nc.vector.scalar_tensor_tensor(
                h, in0=E, scalar=-1.0, in1=R, op0=ALU.add, op1=ALU.min
            )
            H1.append(h)
            if i % 2 == 1:
                yield

        H2 = []
        for j in range(4):
            import os as _os
            _l2g = _os.environ.get("K_L2G", "dve")
            if _l2g == "odd":
                eng = nc.gpsimd if j % 2 == 1 else nc.vector
            elif _l2g == "j3":
                eng = nc.gpsimd if j == 3 else nc.vector
            else:
                eng = nc.vector
            cur = mlp.tile([P, W], F16, tag="u")
            eng.tensor_scalar(
                out=cur, in0=H1[0], scalar1=W2[0][j], scalar2=b2[j],
                op0=ALU.mult, op1=ALU.add,
            )
            for i in range(1, 4):
                nxt = mlp.tile([P, W], F16, tag="u")
                eng.scalar_tensor_tensor(
                    nxt, in0=H1[i], scalar=W2[i][j], in1=cur,
                    op0=ALU.mult, op1=ALU.add,
                )
                cur = nxt
            E = mlp.tile([P, W], F16, tag="e")
            R = mlp.tile([P, W], F16, tag="r")
            nc.scalar.activation(E, cur, AF.Exp, bias=cbias(0.0))
            nc.scalar.activation(R, cur, AF.Relu, bias=cbias(0.0))
            h = mlp.tile([P, W], F16, tag=f"h2_{j}", bufs=1)
            nc.vector.scalar_tensor_tensor(
                h, in0=E, scalar=-1.0, in1=R, op0=ALU.add, op1=ALU.min
            )
            H2.append(h)
            yield

        import os as _os
        _l3 = nc.gpsimd if _os.environ.get("K_L3", "dve") == "gps" else nc.vector
        cur = mlp.tile([P, W], F16, tag="u")
        _l3.tensor_scalar(
            out=cur, in0=H2[0], scalar1=wp2[0], scalar2=bp2,
            op0=ALU.mult, op1=ALU.add,
        )
        for j in range(1, 4):
            nxt = mlp.tile([P, W], F16, tag="u")
            _l3.scalar_tensor_tensor(
                nxt, in0=H2[j], scalar=wp2[j], in1=cur, op0=ALU.mult, op1=ALU.add
            )
            cur = nxt
        eneg = mlp.tile([P, W], F16, tag="e")
        nc.scalar.activation(eneg, cur, AF.Exp, bias=cbias(0.0), scale=-1.0)
        wden = mlp.tile([P, W], F32, tag="u")
        nc.vector.tensor_scalar_add(wden, eneg, 1.0)
        a_out = mlp.tile([P, W], F32, tag="a_out")
        nc.vector.reciprocal(a_out, wden)
        yield

        # A01 = A10^T  (att pre-normalization is symmetric)
        t01p = pp_mlp.tile([P, 3 * P], F32, tag="mlpT")
        nc.tensor.transpose(t01p[:, 0:P], a_out[:, P:2 * P], ident)
        t01 = small.tile([P, P], F32, tag="t01")
        S01 = small.tile([P, 1], F32, tag="s01")
        nc.scalar.copy(t01, t01p[:, 0:P])
        nc.vector.tensor_reduce(S01, t01, axis=X_AX, op=ALU.add)

        # column sums via row sums (symmetry)
        Sa = small.tile([P, 1], F32, tag="sa")
        nc.vector.tensor_reduce(Sa, a_out[:, 0:P], axis=X_AX, op=ALU.add)
        Sc0 = small.tile([P, 1], F32, tag="sc0")
        nc.vector.tensor_add(Sc0, Sa, S01)
        Sc1 = small.tile([P, 1], F32, tag="sc1")
        nc.vector.tensor_reduce(Sc1, a_out[:, P:3 * P], axis=X_AX, op=ALU.add)
        yield
        R0 = small.tile([P, 1], F32, tag="r0")
        R1 = small.tile([P, 1], F32, tag="r1")
        t0 = small.tile([P, 1], F32, tag="t0")
        t1 = small.tile([P, 1], F32, tag="t1")
        nc.vector.tensor_scalar_add(t0, Sc0, EPS)
        nc.vector.tensor_scalar_add(t1, Sc1, EPS)
        nc.vector.reciprocal(R0, t0)
        nc.vector.reciprocal(R1, t1)

        # U = R * (A * offdiag-mask), blockwise
        n00 = small.tile([P, P], F32, tag="n00")
        n11 = small.tile([P, P], F32, tag="n11")
        nc.vector.tensor_mul(n00, a_out[:, 0:P], od)
        nc.vector.tensor_mul(n11, a_out[:, 2 * P:3 * P], od)
        v00 = small.tile([P, P], F32, tag="v00")
        v01 = small.tile([P, P], F32, tag="v01")
        v10 = small.tile([P, P], F32, tag="v10")
        v11 = small.tile([P, P], F32, tag="v11")
        nc.vector.tensor_scalar_mul(v00, n00, R0)
        nc.vector.tensor_scalar_mul(v01, t01, R0)
        nc.vector.tensor_scalar_mul(v10, a_out[:, P:2 * P], R1)
        nc.vector.tensor_scalar_mul(v11, n11, R1)

        # A'' = U^T + I
        p0 = pp_mlp.tile([P, 3 * P], F32, tag="mlpT")
        nc.tensor.transpose(p0[:, 0:P], v00, ident)
        nc.tensor.transpose(p0[:, P:2 * P], v10, ident)
        p1 = pp_mlp.tile([P, 3 * P], F32, tag="mlpT")
        nc.tensor.transpose(p1[:, 0:P], v01, ident)
        nc.tensor.transpose(p1[:, P:2 * P], v11, ident)
        A0 = apool.tile([P, C], F32R, tag="A0")
        A1 = apool.tile([P, C], F32R, tag="A1")
        nc.vector.scalar_tensor_tensor(
            A0[:, 0:P], in0=p0[:, 0:P], scalar=1.0, in1=ident,
            op0=ALU.mult, op1=ALU.add,
        )
        nc.scalar.copy(A0[:, P:C], p0[:, P:2 * P])
        nc.scalar.copy(A1[:, 0:P], p1[:, 0:P])
        nc.vector.scalar_tensor_tensor(
            A1[:, P:C], in0=p1[:, P:2 * P], scalar=1.0, in1=ident,
            op0=ALU.mult, op1=ALU.add,
        )
        st.A = (A0, A1)

    # ---------------- stage C: out = X @ A'' ----------------
    def stage_c(b, st):
        A0, A1 = st.A
        XT0, XT1 = st.XT
        for g in range(NCHUNK):
            ost = ostage.tile([P, LT, C], F32, tag="ost")
            for lt in range(LT):
                yield
                tb = g * LT + lt
                po = pp_out.tile([P, C], F32, tag="po")
                nc.tensor.matmul(
                    po, XT0[:, tb * P:(tb + 1) * P], A0,
                    start=True, stop=False,
                )
                nc.tensor.matmul(
                    po, XT1[:, tb * P:(tb + 1) * P], A1,
                    start=False, stop=True,
                )
                if lt % 2 == 0:
                    nc.scalar.copy(ost[:, lt, :], po)
                else:
                    nc.vector.tensor_copy(ost[:, lt, :], po)
            dst = o_d[b, g * LT * P:(g + 1) * LT * P, :].rearrange(
                "(lt p) c -> p lt c", p=P
            )
            nc.sync.dma_start(out=dst, in_=ost)

    # ---------------- software-pipelined emission ----------------
    import os as _os
    nb = int(_os.environ.get("KERNEL_NBATCH", str(B_PER_CORE)))
    _ = _os
    sts = [_St() for _ in range(B_PER_CORE)]
    del STAGE_RANGES[:]
    _ga, _gb, _gc = stage_a, stage_b, stage_c

    def drive(*gens):
        live = list(gens)
        while live:
            for g in list(live):
                try:
                    next(g)
                except StopIteration:
                    live.remove(g)

    def _tag(label, gen):
        _mark(nc, label)
        return gen

    def stage_a(b, st):
        _mark(nc, f"A{b}")
        drive(_ga(b, st))

    def stage_b(b, st):
        _mark(nc, f"B{b}")
        drive(_gb(b, st))

    def stage_c(b, st):
        _mark(nc, f"C{b}")
        drive(_gc(b, st))

    if nb == 1:
        stage_a(0, sts[0]); stage_b(0, sts[0]); stage_c(0, sts[0])
    elif nb == 2:
        stage_a(0, sts[0]); stage_a(1, sts[1])
        stage_b(0, sts[0]); stage_c(0, sts[0])
        stage_b(1, sts[1]); stage_c(1, sts[1])
    elif _os.environ.get("K_PIPE", "i") == "i":
        # op-granular interleave: fill MLP-chain stalls with bulk work
        _mark(nc, "A0")
        drive(_ga(0, sts[0]))
        _mark(nc, "A1B0")
        drive(_gb(0, sts[0]), _ga(1, sts[1]))
        _mark(nc, "B1C0A2")
        drive(_gb(1, sts[1]), _gc(0, sts[0]), _ga(2, sts[2]))
        _mark(nc, "B2C1A3")
        drive(_gb(2, sts[2]), _gc(1, sts[1]), _ga(3, sts[3]))
        _mark(nc, "B3C2")
        drive(_gb(3, sts[3]), _gc(2, sts[2]))
        _mark(nc, "C3")
        drive(_gc(3, sts[3]))
    elif _os.environ.get("K_PIPE", "i") == "3":
        stage_a(0, sts[0])
        stage_a(1, sts[1])
        stage_b(0, sts[0])
        stage_a(2, sts[2])
        stage_c(0, sts[0])
        stage_b(1, sts[1])
        stage_a(3, sts[3])
        stage_c(1, sts[1])
        stage_b(2, sts[2])
        stage_c(2, sts[2])
        stage_b(3, sts[3])
        stage_c(3, sts[3])
    else:
        stage_a(0, sts[0])
        stage_a(1, sts[1])
        stage_b(0, sts[0])
        stage_c(0, sts[0])
        stage_a(2, sts[2])
        stage_b(1, sts[1])
        stage_c(1, sts[1])
        stage_a(3, sts[3])
        stage_b(2, sts[2])
        stage_c(2, sts[2])
        stage_b(3, sts[3])
        stage_c(3, sts[3])

    ctx.close()


_ACT_TABLES_PATCHED = False


def _pin_act_table():
    """Force the act-table chooser onto natural_log_exp_and_others (which
    covers Copy/Ln/Exp/Relu) so the whole kernel needs ONE table load
    instead of thrashing between exp_and_others and natural_log."""
    global _ACT_TABLES_PATCHED
    if _ACT_TABLES_PATCHED:
        return
    from concourse import hw_specs
    import concourse.bacc as bacc_mod
    orig = hw_specs.get_activation_tables
    mine = {AF.Copy, AF.Ln, AF.Exp, AF.Relu, AF.Identity}
    keep = "natural_log_exp_and_others"

    def patched(arch):
        tabs = orig(arch)
        if keep not in tabs or not mine <= tabs[keep]:
            return tabs
        return {
            name: (s if name == keep else s - mine)
            for name, s in tabs.items()
        }

    bacc_mod.get_activation_tables = patched
    _ACT_TABLES_PATCHED = True


def build_program(inputs):
    """Build + compile the SPMD Bass program with folded params baked in."""
    _pin_act_table()
    w1, b1, W2, b2, wp2, bp2 = _fold_params(inputs)
    nc = bacc.Bacc(
        "TRN2",
        target_bir_lowering=False,
        debug=False,
        enable_asserts=False,
        num_devices=N_CORES,
    )
    x_d = nc.dram_tensor("x", (B_PER_CORE, T, C), F32, kind="ExternalInput").ap()
    o_d = nc.dram_tensor("out", (B_PER_CORE, T, C), F32, kind="ExternalOutput").ap()
    with tile.TileContext(nc) as tc:
        _build_kernel(tc, o_d, x_d, w1, b1, W2, b2, wp2, bp2)
    nc.compile()
    return nc


def run_device(nc, x, trace=False, **kw):
    """Run the compiled program over the 8 cores; return (out, results)."""
    x = np.asarray(x, np.float32)
    in_maps = [
        {"x": np.ascontiguousarray(x[c * B_PER_CORE:(c + 1) * B_PER_CORE])}
        for c in range(N_CORES)
    ]
    res = bass_utils.run_bass_kernel_spmd(
        nc, in_maps, core_ids=list(range(N_CORES)), trace=trace, **kw
    )
    out = np.concatenate(
        [res.results[c]["out"] for c in range(N_CORES)], axis=0
    )
    return out, res


def host_penalty(inputs):
    """Monotonicity penalty on 21 keypoints; float32 math mirroring reference."""
    f32 = np.float32
    W1 = np.asarray(inputs["W1"], f32)
    b1 = np.asarray(inputs["b1"], f32)
    W2 = np.asarray(inputs["W2"], f32)
    b2 = np.asarray(inputs["b2"], f32)
    gamma = np.asarray(inputs["gamma"], f32)
    beta = np.asarray(inputs["beta"], f32)
    mmean = np.asarray(inputs["mmean"], f32)
    mvar = np.asarray(inputs["mvar"], f32)
    Wp = np.asarray(inputs["Wp"], f32)
    bp = np.asarray(inputs["bp"], f32)

    def elu(v):
        return np.where(v > 0, v, np.expm1(v)).astype(f32)

    z = np.linspace(-1.0, 1.0, 21).astype(f32).reshape(-1, 1)
    h = elu(z @ W1 + b1)
    h = elu(h @ W2 + b2)
    h = (h - mmean) * (f32(1.0) / np.sqrt(mvar + f32(BN_EPS))) * gamma + beta
    o = h @ Wp + bp
    kout = (f32(1.0) / (f32(1.0) + np.exp(-o)))[:, 0]
    dL = kout[1:11] - kout[:10]
    dR = kout[11:] - kout[10:-1]
    pen = f32(0.5) * f32(PENALTY_RATE) * np.mean(
        np.abs(dL) - dL + np.abs(dR) - dR, dtype=f32
    )
    return f32(pen)


def kernel(**inputs):
    x = np.asarray(inputs["x"], np.float32)
    nc = build_program(inputs)
    out, _ = run_device(nc, x)
    penalty = host_penalty(inputs)
    return out, penalty


# revision 36
# speedup vs baseline: 36626.7850x; 1.0032x over previous
"""Trainium2 Bass kernel for nn_FC_mono_12086037971055 (dense_mlp).

Computation (per batch b of x: (T=4096, C=256)):
  norm_x = x / sqrt(sum_t x^2 + 1e-7)          (column-normalize over T)
  cor    = norm_x^T @ norm_x                   (C x C Gram of correlations)
  att    = MLP(cor) elementwise                (1->4 elu ->4 elu -> BN -> 1 sigmoid)
  att    = att / (sum_axis1(att) + 1e-7)       (column-normalize)
  out    = x + x @ (offdiag * att)
plus a scalar monotonicity penalty from 21 keypoints (computed host-side: it
depends only on the tiny MLP params).

Sharding: data-parallel over batch B=32 -> 4 batches on each of 8 cores.
All MLP/BN parameters are folded on the host into scalar immediates that are
baked into the Bass program (BN is affine at inference, so it folds into the
final dense layer).

Kernel strategy per batch (one NeuronCore):
  - G = X^T X with raw X via PE matmul in float32r (full rate at N=256).
    Column norms Q_c are G's diagonal, extracted with an eye-mask reduce.
  - cor = s_c * s_d * G with s = 1/sqrt(Q+eps).  The per-free-dim scale is
    applied via PE transposes of row-scaled blocks (G is symmetric, so only
    blocks 00, 10, 11 are needed; 10 comes from transposing row-scaled 01).
  - MLP runs elementwise on one packed (128, 384) tile.
    elu(u) = min(exp(u) - 1, relu(u)), exp/relu/sigmoid on ScalarE.
  - att column-normalize: row-sums (symmetry) -> R = 1/(S+eps); the
    per-column application is again row-scale + PE transpose.  The offdiag
    mask and the +Identity fold (out = X @ (offdiag*att + I)) are applied
    during the PSUM->SBUF copies.
  - out rows: lhsT = X^T (built with PE transposes of X), rhs = A''.
"""

import numpy as np

import concourse.bass as bass
import concourse.tile as tile
from concourse import bacc, bass_utils, mybir

F32 = mybir.dt.float32
F32R = mybir.dt.float32r
F16 = mybir.dt.float16
AF = mybir.ActivationFunctionType
ALU = mybir.AluOpType
X_AX = mybir.AxisListType.X

N_CORES = 8
B_TOTAL = 32
B_PER_CORE = B_TOTAL // N_CORES   # 4
T = 4096
C = 256
P = 128
NT = T // P                        # 32 t-blocks
NCHUNK = 8                         # x streamed in 8 chunks per batch
LT = NT // NCHUNK                  # 8 t-blocks per chunk
BN_EPS = 1e-3
EPS = 1e-7
PENALTY_RATE = 10.0


def _fold_params(inputs):
    """Fold BN into the projection layer; return plain python floats."""
    W1 = np.asarray(inputs["W1"], np.float64)     # (1, 4)
    b1 = np.asarray(inputs["b1"], np.float64)     # (4,)
    W2 = np.asarray(inputs["W2"], np.float64)     # (4, 4)
    b2 = np.asarray(inputs["b2"], np.float64)     # (4,)
    gamma = np.asarray(inputs["gamma"], np.float64)
    beta = np.asarray(inputs["beta"], np.float64)
    mmean = np.asarray(inputs["mmean"], np.float64)
    mvar = np.asarray(inputs["mvar"], np.float64)
    Wp = np.asarray(inputs["Wp"], np.float64)     # (4, 1)
    bp = np.asarray(inputs["bp"], np.float64)     # (1,)

    a = gamma / np.sqrt(mvar + BN_EPS)
    wp2 = Wp[:, 0] * a
    bp2 = bp[0] + np.sum(Wp[:, 0] * (beta - mmean * a))
    return (
        [float(v) for v in W1[0]],
        [float(v) for v in b1],
        [[float(W2[i, j]) for j in range(4)] for i in range(4)],
        [float(v) for v in b2],
        [float(v) for v in wp2],
        float(bp2),
    )


class _St:
    pass


STAGE_RANGES = []


def _mark(nc, label):
    STAGE_RANGES.append((label, int(nc.next_id())))


def _build_kernel(tc, o_d, x_d, w1, b1, W2, b2, wp2, bp2):
    nc = tc.nc

    import contextlib
    ctx = contextlib.ExitStack()
    consts = ctx.enter_context(tc.tile_pool(name="consts", bufs=1))
    xin = ctx.enter_context(tc.tile_pool(name="xin", bufs=6))
    xrp = ctx.enter_context(tc.tile_pool(name="xrp", bufs=6))
    xtp = ctx.enter_context(tc.tile_pool(name="xtp", bufs=6))
    mlp = ctx.enter_context(tc.tile_pool(name="mlp", bufs=3))
    small = ctx.enter_context(tc.tile_pool(name="small", bufs=2))
    apool = ctx.enter_context(tc.tile_pool(name="apool", bufs=2))
    ostage = ctx.enter_context(tc.tile_pool(name="ostage", bufs=4))
    pp_gram = ctx.enter_context(tc.tile_pool(name="pp_gram", bufs=1, space="PSUM"))
    pp_xt = ctx.enter_context(tc.tile_pool(name="pp_xt", bufs=2, space="PSUM"))
    pp_mlp = ctx.enter_context(tc.tile_pool(name="pp_mlp", bufs=1, space="PSUM"))
    pp_out = ctx.enter_context(tc.tile_pool(name="pp_out", bufs=3, space="PSUM"))

    # constants: identity (also the eye mask) and offdiag = 1 - eye
    from concourse.masks import make_identity
    ident = consts.tile([P, P], F32, tag="ident")
    make_identity(nc, ident)
    identr = consts.tile([P, P], F32R, tag="identr")
    nc.vector.tensor_copy(identr, ident)
    od = consts.tile([P, P], F32, tag="od")
    nc.vector.tensor_scalar(
        out=od, in0=ident, scalar1=-1.0, scalar2=1.0, op0=ALU.mult, op1=ALU.add
    )

    # (128,1) constant tiles used as activation biases
    bias_tiles = {}

    def cbias(val):
        v = float(val)
        if v not in bias_tiles:
            t = consts.tile(
                [P, 1], F32, tag=f"cb{len(bias_tiles)}",
                name=f"cb{len(bias_tiles)}",
            )
            nc.vector.memset(t, v)
            bias_tiles[v] = t
        return bias_tiles[v]

    def r32(ap):
        return ap.bitcast(F32R)

    # PE warmup: keep TensorE busy through the initial DMA fill so the
    # first real Gram matmuls run at full clock (p-state ramp done).
    warm = pp_mlp.tile([P, P], F32, tag="mlpT", name="warm")
    for _ in range(36):
        nc.tensor.matmul(warm, identr, identr, start=True, stop=True)

    # ---------------- stage A: load, Gram, X^T, cor -> m_in ----------------
    def stage_a(b, st):
        # load x[b] in 4 chunks of (1024, 256)
        st.X = []
        for g in range(NCHUNK):
            xg = xin.tile([P, LT, C], F32, tag="xin")
            src = x_d[b, g * LT * P:(g + 1) * LT * P, :].rearrange(
                "(lt p) c -> p lt c", p=P
            )
            nc.sync.dma_start(out=xg, in_=src)
            # round to f32r for full-rate PE consumption (GPSIMD is idle)
            xr = xrp.tile([P, LT, C], F32R, tag="xr")
            nc.gpsimd.tensor_copy(xr, xg)
            st.X.append(xr)

        st.XT = [
            xtp.tile([P, T], F32R, tag="xt", name=f"xt_b{b}_c{cb}")
            for cb in range(2)
        ]

        G0 = pp_gram.tile([P, C], F32, tag="g0")
        G1 = pp_gram.tile([P, C], F32, tag="g1")

        for g in range(NCHUNK):
            yield
            xg = st.X[g]
            for lt in range(LT):
                k = g * LT + lt
                xa = xg[:, lt, :]
                nc.tensor.matmul(
                    G0[:, :], xa[:, 0:P], xa,
                    start=(k == 0), stop=(k == NT - 1),
                )
                nc.tensor.matmul(
                    G1[:, :], xa[:, P:C], xa,
                    start=(k == 0), stop=(k == NT - 1),
                )
            # transposes of this chunk into X^T
            for cb in range(2):
                for h in range(LT // 4):
                    tg = pp_xt.tile([P, 4 * P], F32R, tag="xtT")
                    for i in range(4):
                        lt = h * 4 + i
                        nc.tensor.transpose(
                            tg[:, i * P:(i + 1) * P],
                            xg[:, lt, cb * P:(cb + 1) * P],
                            identr,
                        )
                    t0 = (g * LT + h * 4) * P
                    dst = st.XT[cb][:, t0:t0 + 4 * P]
                    if (g * 2 + cb + h) % 4 != 3:
                        nc.scalar.copy(dst, tg)
                    else:
                        nc.vector.tensor_copy(dst, tg)

        # copy G out of PSUM immediately so the next batch's Gram can start
        Gs0 = small.tile([P, C], F32, tag="gs0")
        Gs1 = small.tile([P, P], F32, tag="gs1")
        nc.scalar.copy(Gs0, G0)
        nc.scalar.copy(Gs1, G1[:, P:C])

        # column norms from the Gram diagonal
        scr = small.tile([P, P], F32, tag="scr")
        Q0 = small.tile([P, 1], F32, tag="q0")
        Q1 = small.tile([P, 1], F32, tag="q1")
        nc.vector.tensor_mul(scr, Gs0[:, 0:P], ident)
        nc.vector.tensor_reduce(Q0, scr, axis=X_AX, op=ALU.add)
        scr2 = small.tile([P, P], F32, tag="scr2")
        nc.vector.tensor_mul(scr2, Gs1, ident)
        nc.vector.tensor_reduce(Q1, scr2, axis=X_AX, op=ALU.add)
        sq0 = small.tile([P, 1], F32, tag="sq0")
        sq1 = small.tile([P, 1], F32, tag="sq1")
        nc.scalar.activation(sq0, Q0, AF.Ln, bias=cbias(EPS), scale=1.0)
        nc.scalar.activation(sq1, Q1, AF.Ln, bias=cbias(EPS), scale=1.0)
        s0 = small.tile([P, 1], F32, tag="s0")
        s1 = small.tile([P, 1], F32, tag="s1")
        nc.scalar.activation(s0, sq0, AF.Exp, bias=cbias(0.0), scale=-0.5)
        nc.scalar.activation(s1, sq1, AF.Exp, bias=cbias(0.0), scale=-0.5)

        # cor blocks: row-scale, transpose, scale again on the copy out
        u00 = small.tile([P, P], F32, tag="u00")
        u01 = small.tile([P, P], F32, tag="u01")
        u11 = small.tile([P, P], F32, tag="u11")
        nc.vector.tensor_scalar_mul(u00, Gs0[:, 0:P], s0)
        nc.vector.tensor_scalar_mul(u01, Gs0[:, P:C], s0)
        nc.vector.tensor_scalar_mul(u11, Gs1, s1)
        tgc = pp_mlp.tile([P, 3 * P], F32, tag="mlpT")
        nc.tensor.transpose(tgc[:, 0:P], u00, ident)
        nc.tensor.transpose(tgc[:, P:2 * P], u01, ident)
        nc.tensor.transpose(tgc[:, 2 * P:3 * P], u11, ident)
        m_in = mlp.tile([P, 3 * P], F32, tag="m_in", bufs=3)
        nc.vector.tensor_scalar_mul(m_in[:, 0:P], tgc[:, 0:P], s0)
        nc.vector.tensor_scalar_mul(m_in[:, P:2 * P], tgc[:, P:2 * P], s1)
        nc.vector.tensor_scalar_mul(m_in[:, 2 * P:3 * P], tgc[:, 2 * P:3 * P], s1)
        st.m_in = m_in

    # ---------------- stage B: MLP + normalize -> A'' ----------------
    def stage_b(b, st):
        m_in = st.m_in
        W = 3 * P

        def elu_pair(src, scale, bias):
            E = mlp.tile([P, W], F32, tag="e")
            R = mlp.tile([P, W], F32, tag="r")
            nc.scalar.activation(E, src, AF.Exp, bias=bias, scale=scale)
            nc.scalar.activation(R, src, AF.Relu, bias=bias, scale=scale)
            h = mlp.tile([P, W], F32, tag="h")
            nc.vector.scalar_tensor_tensor(
                h, in0=E, scalar=-1.0, in1=R, op0=ALU.add, op1=ALU.min
            )
            return h

        H1 = []
        for i in range(4):
            E = mlp.tile([P, W], F16, tag="e")
            R = mlp.tile([P, W], F16, tag="r")
            nc.scalar.activation(E, m_in, AF.Exp, bias=cbias(b1[i]), scale=w1[i])
            import os as _os
            if b1[i] == 0.0 and _os.environ.get("K_R1", "act") == "dve":
                nc.vector.tensor_scalar(
                    out=R, in0=m_in, scalar1=w1[i], scalar2=0.0,
                    op0=ALU.mult, op1=ALU.max,
                )
            else:
                nc.scalar.activation(R, m_in, AF.Relu, bias=cbias(b1[i]), scale=w1[i])
            h = mlp.tile([P, W], F16, tag=f"h1_{i}", bufs=1)
            nc.vector.scalar_tensor_tensor(
                h, in0=E, scalar=-1.0, in1=R, op0=ALU.add, op1=ALU.min
            )
            H1.append(h)
            if i % 2 == 1:
                yield

        H2 = []
        for j in range(4):
            import os as _os
            _l2g = _os.environ.get("K_L2G", "dve")
            if _l2g == "odd":
                eng = nc.gpsimd if j % 2 == 1 else nc.vector
            elif _l2g == "j3":
                eng = nc.gpsimd if j == 3 else nc.vector
            else:
                eng = nc.vector
            cur = mlp.tile([P, W], F16, tag="u")
            eng.tensor_scalar(
                out=cur, in0=H1[0], scalar1=W2[0][j], scalar2=b2[j],
                op0=ALU.mult, op1=ALU.add,
            )
            for i in range(1, 4):
                nxt = mlp.tile([P, W], F16, tag="u")
                eng.scalar_tensor_tensor(
                    nxt, in0=H1[i], scalar=W2[i][j], in1=cur,
                    op0=ALU.mult, op1=ALU.add,
                )
                cur = nxt
            E = mlp.tile([P, W], F16, tag="e")
            R = mlp.tile([P, W], F16, tag="r")
            nc.scalar.activation(E, cur, AF.Exp, bias=cbias(0.0))
            nc.scalar.activation(R, cur, AF.Relu, bias=cbias(0.0))
            h = mlp.tile([P, W], F16, tag=f"h2_{j}", bufs=1)
            nc.vector.scalar_tensor_tensor(
                h, in0=E, scalar=-1.0, in1=R, op0=ALU.add, op1=ALU.min
            )
            H2.append(h)
            yield

        import os as _os
        _l3 = nc.gpsimd if _os.environ.get("K_L3", "dve") == "gps" else nc.vector
        cur = mlp.tile([P, W], F16, tag="u")
        _l3.tensor_scalar(
            out=cur, in0=H2[0], scalar1=wp2[0], scalar2=bp2,
            op0=ALU.mult, op1=ALU.add,
        )
        for j in range(1, 4):
            nxt = mlp.tile([P, W], F16, tag="u")
            _l3.scalar_tensor_tensor(
                nxt, in0=H2[j], scalar=wp2[j], in1=cur, op0=ALU.mult, op1=ALU.add
            )
            cur = nxt
        eneg = mlp.tile([P, W], F16, tag="e")
        nc.scalar.activation(eneg, cur, AF.Exp, bias=cbias(0.0), scale=-1.0)
        wden = mlp.tile([P, W], F32, tag="u")
        nc.vector.tensor_scalar_add(wden, eneg, 1.0)
        a_out = mlp.tile([P, W], F32, tag="a_out")
        nc.vector.reciprocal(a_out, wden)
        yield

        # A01 = A10^T  (att pre-normalization is symmetric)
        t01p = pp_mlp.tile([P, 3 * P], F32, tag="mlpT")
        nc.tensor.transpose(t01p[:, 0:P], a_out[:, P:2 * P], ident)
        t01 = small.tile([P, P], F32, tag="t01")
        S01 = small.tile([P, 1], F32, tag="s01")
        nc.scalar.copy(t01, t01p[:, 0:P])
        nc.vector.tensor_reduce(S01, t01, axis=X_AX, op=ALU.add)

        # column sums via row sums (symmetry)
        Sa = small.tile([P, 1], F32, tag="sa")
        nc.vector.tensor_reduce(Sa, a_out[:, 0:P], axis=X_AX, op=ALU.add)
        Sc0 = small.tile([P, 1], F32, tag="sc0")
        nc.vector.tensor_add(Sc0, Sa, S01)
        Sc1 = small.tile([P, 1], F32, tag="sc1")
        nc.vector.tensor_reduce(Sc1, a_out[:, P:3 * P], axis=X_AX, op=ALU.add)
        yield
        R0 = small.tile([P, 1], F32, tag="r0")
        R1 = small.tile([P, 1], F32, tag="r1")
        t0 = small.tile([P, 1], F32, tag="t0")
        t1 = small.tile([P, 1], F32, tag="t1")
        nc.vector.tensor_scalar_add(t0, Sc0, EPS)
        nc.vector.tensor_scalar_add(t1, Sc1, EPS)
        nc.vector.reciprocal(R0, t0)
        nc.vector.reciprocal(R1, t1)

        # U = R * (A * offdiag-mask), blockwise
        n00 = small.tile([P, P], F32, tag="n00")
        n11 = small.tile([P, P], F32, tag="n11")
        nc.vector.tensor_mul(n00, a_out[:, 0:P], od)
        nc.vector.tensor_mul(n11, a_out[:, 2 * P:3 * P], od)
        v00 = small.tile([P, P], F32, tag="v00")
        v01 = small.tile([P, P], F32, tag="v01")
        v10 = small.tile([P, P], F32, tag="v10")
        v11 = small.tile([P, P], F32, tag="v11")
        nc.vector.tensor_scalar_mul(v00, n00, R0)
        nc.vector.tensor_scalar_mul(v01, t01, R0)
        nc.vector.tensor_scalar_mul(v10, a_out[:, P:2 * P], R1)
        nc.vector.tensor_scalar_mul(v11, n11, R1)

        # A'' = U^T + I
        p0 = pp_mlp.tile([P, 3 * P], F32, tag="mlpT")
        nc.tensor.transpose(p0[:, 0:P], v00, ident)
        nc.tensor.transpose(p0[:, P:2 * P], v10, ident)
        p1 = pp_mlp.tile([P, 3 * P], F32, tag="mlpT")
        nc.tensor.transpose(p1[:, 0:P], v01, ident)
        nc.tensor.transpose(p1[:, P:2 * P], v11, ident)
        A0 = apool.tile([P, C], F32R, tag="A0")
        A1 = apool.tile([P, C], F32R, tag="A1")
        nc.vector.scalar_tensor_tensor(
            A0[:, 0:P], in0=p0[:, 0:P], scalar=1.0, in1=ident,
            op0=ALU.mult, op1=ALU.add,
        )
        nc.scalar.copy(A0[:, P:C], p0[:, P:2 * P])
        nc.scalar.copy(A1[:, 0:P], p1[:, 0:P])
        nc.vector.scalar_tensor_tensor(
            A1[:, P:C], in0=p1[:, P:2 * P], scalar=1.0, in1=ident,
            op0=ALU.mult, op1=ALU.add,
        )
        st.A = (A0, A1)

    # ---------------- stage C: out = X @ A'' ----------------
    def stage_c(b, st):
        A0, A1 = st.A
        XT0, XT1 = st.XT
        for g in range(NCHUNK):
            ost = ostage.tile([P, LT, C], F32, tag="ost")
            for lt in range(LT):
                yield
                tb = g * LT + lt
                po = pp_out.tile([P, C], F32, tag="po")
                nc.tensor.matmul(
                    po, XT0[:, tb * P:(tb + 1) * P], A0,
                    start=True, stop=False,
                )
                nc.tensor.matmul(
                    po, XT1[:, tb * P:(tb + 1) * P], A1,
                    start=False, stop=True,
                )
                if lt % 2 == 0:
                    nc.scalar.copy(ost[:, lt, :], po)
                else:
                    nc.vector.tensor_copy(ost[:, lt, :], po)
            dst = o_d[b, g * LT * P:(g + 1) * LT * P, :].rearrange(
                "(lt p) c -> p lt c", p=P
            )
            nc.sync.dma_start(out=dst, in_=ost)

    # ---------------- software-pipelined emission ----------------
    import os as _os
    nb = int(_os.environ.get("KERNEL_NBATCH", str(B_PER_CORE)))
    _ = _os
    sts = [_St() for _ in range(B_PER_CORE)]
    del STAGE_RANGES[:]
    _ga, _gb, _gc = stage_a, stage_b, stage_c

    def drive(*gens):
        live = list(gens)
        while live:
            for g in list(live):
                try:
                    next(g)
                except StopIteration:
                    live.remove(g)

    def _tag(label, gen):
        _mark(nc, label)
        return gen

    def stage_a(b, st):
        _mark(nc, f"A{b}")
        drive(_ga(b, st))

    def stage_b(b, st):
        _mark(nc, f"B{b}")
        drive(_gb(b, st))

    def stage_c(b, st):
        _mark(nc, f"C{b}")
        drive(_gc(b, st))

    if nb == 1:
        stage_a(0, sts[0]); stage_b(0, sts[0]); stage_c(0, sts[0])
    elif nb == 2:
        stage_a(0, sts[0]); stage_a(1, sts[1])
        stage_b(0, sts[0]); stage_c(0, sts[0])
        stage_b(1, sts[1]); stage_c(1, sts[1])
    elif _os.environ.get("K_PIPE", "i") == "i":
        # op-granular interleave: fill MLP-chain stalls with bulk work
        _mark(nc, "A0")
        drive(_ga(0, sts[0]))
        _mark(nc, "A1B0")
        drive(_gb(0, sts[0]), _ga(1, sts[1]))
        _mark(nc, "B1C0A2")
        drive(_gb(1, sts[1]), _gc(0, sts[0]), _ga(2, sts[2]))
        _mark(nc, "B2C1A3")
        drive(_gb(2, sts[2]), _gc(1, sts[1]), _ga(3, sts[3]))
        _mark(nc, "B3C2")
        drive(_gb(3, sts[3]), _gc(2, sts[2]))
        _mark(nc, "C3")
        drive(_gc(3, sts[3]))
    elif _os.environ.get("K_PIPE", "i") == "3":
        stage_a(0, sts[0])
        stage_a(1, sts[1])
        stage_b(0, sts[0])
        stage_a(2, sts[2])
        stage_c(0, sts[0])
        stage_b(1, sts[1])
        stage_a(3, sts[3])
        stage_c(1, sts[1])
        stage_b(2, sts[2])
        stage_c(2, sts[2])
        stage_b(3, sts[3])
        stage_c(3, sts[3])
    else:
        stage_a(0, sts[0])
        stage_a(1, sts[1])
        stage_b(0, sts[0])
        stage_c(0, sts[0])
        stage_a(2, sts[2])
        stage_b(1, sts[1])
        stage_c(1, sts[1])
        stage_a(3, sts[3])
        stage_b(2, sts[2])
        stage_c(2, sts[2])
        stage_b(3, sts[3])
        stage_c(3, sts[3])

    ctx.close()


_ACT_TABLES_PATCHED = False


def _pin_act_table():
    """Force the act-table chooser onto natural_log_exp_and_others (which
    covers Copy/Ln/Exp/Relu) so the whole kernel needs ONE table load
    instead of thrashing between exp_and_others and natural_log."""
    global _ACT_TABLES_PATCHED
    if _ACT_TABLES_PATCHED:
        return
    from concourse import hw_specs
    import concourse.bacc as bacc_mod
    orig = hw_specs.get_activation_tables
    mine = {AF.Copy, AF.Ln, AF.Exp, AF.Relu, AF.Identity}
    keep = "natural_log_exp_and_others"

    def patched(arch):
        tabs = orig(arch)
        if keep not in tabs or not mine <= tabs[keep]:
            return tabs
        return {
            name: (s if name == keep else s - mine)
            for name, s in tabs.items()
        }

    bacc_mod.get_activation_tables = patched
    _ACT_TABLES_PATCHED = True


def build_program(inputs):
    """Build + compile the SPMD Bass program with folded params baked in."""
    _pin_act_table()
    w1, b1, W2, b2, wp2, bp2 = _fold_params(inputs)
    nc = bacc.Bacc(
        "TRN2",
        target_bir_lowering=False,
        debug=False,
        enable_asserts=False,
        num_devices=N_CORES,
    )
    x_d = nc.dram_tensor("x", (B_PER_CORE, T, C), F32, kind="ExternalInput").ap()
    o_d = nc.dram_tensor("out", (B_PER_CORE, T, C), F32, kind="ExternalOutput").ap()
    with tile.TileContext(nc) as tc:
        _build_kernel(tc, o_d, x_d, w1, b1, W2, b2, wp2, bp2)
    nc.compile()
    return nc


def run_device(nc, x, trace=False, **kw):
    """Run the compiled program over the 8 cores; return (out, results)."""
    x = np.asarray(x, np.float32)
    in_maps = [
        {"x": np.ascontiguousarray(x[c * B_PER_CORE:(c + 1) * B_PER_CORE])}
        for c in range(N_CORES)
    ]
    res = bass_utils.run_bass_kernel_spmd(
        nc, in_maps, core_ids=list(range(N_CORES)), trace=trace, **kw
    )
    out = np.concatenate(
        [res.results[c]["out"] for c in range(N_CORES)], axis=0
    )
    return out, res


def host_penalty(inputs):
    """Monotonicity penalty on 21 keypoints; float32 math mirroring reference."""
    f32 = np.float32
    W1 = np.asarray(inputs["W1"], f32)
    b1 = np.asarray(inputs["b1"], f32)
    W2 = np.asarray(inputs["W2"], f32)
    b2 = np.asarray(inputs["b2"], f32)
    gamma = np.asarray(inputs["gamma"], f32)
    beta = np.asarray(inputs["beta"], f32)
    mmean = np.asarray(inputs["mmean"], f32)
    mvar = np.asarray(inputs["mvar"], f32)
    Wp = np.asarray(inputs["Wp"], f32)
    bp = np.asarray(inputs["bp"], f32)

    def elu(v):
        return np.where(v > 0, v, np.expm1(v)).astype(f32)

    z = np.linspace(-1.0, 1.0, 21).astype(f32).reshape(-1, 1)
    h = elu(z @ W1 + b1)
    h = elu(h @ W2 + b2)
    h = (h - mmean) * (f32(1.0) / np.sqrt(mvar + f32(BN_EPS))) * gamma + beta
    o = h @ Wp + bp
    kout = (f32(1.0) / (f32(1.0) + np.exp(-o)))[:, 0]
    dL = kout[1:11] - kout[:10]
    dR = kout[11:] - kout[10:-1]
    pen = f32(0.5) * f32(PENALTY_RATE) * np.mean(
        np.abs(dL) - dL + np.abs(dR) - dR, dtype=f32
    )
    return f32(pen)


def kernel(**inputs):
    x = np.asarray(inputs["x"], np.float32)
    nc = build_program(inputs)
    out, _ = run_device(nc, x)
    penalty = host_penalty(inputs)
    return out, penalty


# revision 41
# speedup vs baseline: 37326.0616x; 1.0191x over previous
"""Trainium2 Bass kernel for nn_FC_mono_12086037971055 (dense_mlp).

Computation (per batch b of x: (T=4096, C=256)):
  norm_x = x / sqrt(sum_t x^2 + 1e-7)          (column-normalize over T)
  cor    = norm_x^T @ norm_x                   (C x C Gram of correlations)
  att    = MLP(cor) elementwise                (1->4 elu ->4 elu -> BN -> 1 sigmoid)
  att    = att / (sum_axis1(att) + 1e-7)       (column-normalize)
  out    = x + x @ (offdiag * att)
plus a scalar monotonicity penalty from 21 keypoints (computed host-side: it
depends only on the tiny MLP params).

Sharding: data-parallel over batch B=32 -> 4 batches on each of 8 cores.
All MLP/BN parameters are folded on the host into scalar immediates that are
baked into the Bass program (BN is affine at inference, so it folds into the
final dense layer).

Kernel strategy per batch (one NeuronCore):
  - G = X^T X with raw X via PE matmul in float32r (full rate at N=256).
    Column norms Q_c are G's diagonal, extracted with an eye-mask reduce.
  - cor = s_c * s_d * G with s = 1/sqrt(Q+eps).  The per-free-dim scale is
    applied via PE transposes of row-scaled blocks (G is symmetric, so only
    blocks 00, 10, 11 are needed; 10 comes from transposing row-scaled 01).
  - MLP runs elementwise on one packed (128, 384) tile.
    elu(u) = min(exp(u) - 1, relu(u)), exp/relu/sigmoid on ScalarE.
  - att column-normalize: row-sums (symmetry) -> R = 1/(S+eps); the
    per-column application is again row-scale + PE transpose.  The offdiag
    mask and the +Identity fold (out = X @ (offdiag*att + I)) are applied
    during the PSUM->SBUF copies.
  - out rows: lhsT = X^T (built with PE transposes of X), rhs = A''.
"""

import numpy as np

import concourse.bass as bass
import concourse.tile as tile
from concourse import bacc, bass_utils, mybir

F32 = mybir.dt.float32
F32R = mybir.dt.float32r
F16 = mybir.dt.float16
AF = mybir.ActivationFunctionType
ALU = mybir.AluOpType
X_AX = mybir.AxisListType.X

N_CORES = 8
B_TOTAL = 32
B_PER_CORE = B_TOTAL // N_CORES   # 4
T = 4096
C = 256
P = 128
NT = T // P                        # 32 t-blocks
NCHUNK = 8                         # x streamed in 8 chunks per batch
LT = NT // NCHUNK                  # 8 t-blocks per chunk
BN_EPS = 1e-3
EPS = 1e-7
PENALTY_RATE = 10.0


def _fold_params(inputs):
    """Fold BN into the projection layer; return plain python floats."""
    W1 = np.asarray(inputs["W1"], np.float64)     # (1, 4)
    b1 = np.asarray(inputs["b1"], np.float64)     # (4,)
    W2 = np.asarray(inputs["W2"], np.float64)     # (4, 4)
    b2 = np.asarray(inputs["b2"], np.float64)     # (4,)
    gamma = np.asarray(inputs["gamma"], np.float64)
    beta = np.asarray(inputs["beta"], np.float64)
    mmean = np.asarray(inputs["mmean"], np.float64)
    mvar = np.asarray(inputs["mvar"], np.float64)
    Wp = np.asarray(inputs["Wp"], np.float64)     # (4, 1)
    bp = np.asarray(inputs["bp"], np.float64)     # (1,)

    a = gamma / np.sqrt(mvar + BN_EPS)
    wp2 = Wp[:, 0] * a
    bp2 = bp[0] + np.sum(Wp[:, 0] * (beta - mmean * a))
    return (
        [float(v) for v in W1[0]],
        [float(v) for v in b1],
        [[float(W2[i, j]) for j in range(4)] for i in range(4)],
        [float(v) for v in b2],
        [float(v) for v in wp2],
        float(bp2),
    )


class _St:
    pass


STAGE_RANGES = []


def _mark(nc, label):
    STAGE_RANGES.append((label, int(nc.next_id())))


def _build_kernel(tc, o_d, x_d, w1, b1, W2, b2, wp2, bp2):
    nc = tc.nc

    import contextlib
    ctx = contextlib.ExitStack()
    consts = ctx.enter_context(tc.tile_pool(name="consts", bufs=1))
    xin = ctx.enter_context(tc.tile_pool(name="xin", bufs=5))
    xrp = ctx.enter_context(tc.tile_pool(name="xrp", bufs=5))
    xtp = ctx.enter_context(tc.tile_pool(name="xtp", bufs=6))
    mlp = ctx.enter_context(tc.tile_pool(name="mlp", bufs=3))
    small = ctx.enter_context(tc.tile_pool(name="small", bufs=2))
    apool = ctx.enter_context(tc.tile_pool(name="apool", bufs=2))
    ostage = ctx.enter_context(tc.tile_pool(name="ostage", bufs=4))
    pp_gram = ctx.enter_context(tc.tile_pool(name="pp_gram", bufs=1, space="PSUM"))
    pp_xt = ctx.enter_context(tc.tile_pool(name="pp_xt", bufs=2, space="PSUM"))
    pp_mlp = ctx.enter_context(tc.tile_pool(name="pp_mlp", bufs=1, space="PSUM"))
    pp_out = ctx.enter_context(tc.tile_pool(name="pp_out", bufs=3, space="PSUM"))

    # constants: identity (also the eye mask) and offdiag = 1 - eye
    from concourse.masks import make_identity
    ident = consts.tile([P, P], F32, tag="ident")
    make_identity(nc, ident)
    identr = consts.tile([P, P], F32R, tag="identr")
    nc.vector.tensor_copy(identr, ident)
    od = consts.tile([P, P], F32, tag="od")
    nc.vector.tensor_scalar(
        out=od, in0=ident, scalar1=-1.0, scalar2=1.0, op0=ALU.mult, op1=ALU.add
    )

    # (128,1) constant tiles used as activation biases
    bias_tiles = {}

    def cbias(val):
        v = float(val)
        if v not in bias_tiles:
            t = consts.tile(
                [P, 1], F32, tag=f"cb{len(bias_tiles)}",
                name=f"cb{len(bias_tiles)}",
            )
            nc.vector.memset(t, v)
            bias_tiles[v] = t
        return bias_tiles[v]

    def r32(ap):
        return ap.bitcast(F32R)

    # PE warmup: keep TensorE busy through the initial DMA fill so the
    # first real Gram matmuls run at full clock (p-state ramp done).
    warm = pp_mlp.tile([P, P], F32, tag="mlpT", name="warm")
    for _ in range(36):
        nc.tensor.matmul(warm, identr, identr, start=True, stop=True)

    # ---------------- stage A: load, Gram, X^T, cor -> m_in ----------------
    def stage_a(b, st):
        # load x[b] in 4 chunks of (1024, 256)
        st.X = []
        for g in range(NCHUNK):
            xg = xin.tile([P, LT, C], F32, tag="xin")
            src = x_d[b, g * LT * P:(g + 1) * LT * P, :].rearrange(
                "(lt p) c -> p lt c", p=P
            )
            nc.sync.dma_start(out=xg, in_=src)
            # round to f32r for full-rate PE consumption (GPSIMD is idle)
            xr = xrp.tile([P, LT, C], F32R, tag="xr")
            nc.gpsimd.tensor_copy(xr, xg)
            st.X.append(xr)

        st.XT = [
            xtp.tile([P, T], F32R, tag="xt", name=f"xt_b{b}_c{cb}")
            for cb in range(2)
        ]

        G0 = pp_gram.tile([P, C], F32, tag="g0")
        G1 = pp_gram.tile([P, C], F32, tag="g1")

        for g in range(NCHUNK):
            yield
            xg = st.X[g]
            for lt in range(LT):
                k = g * LT + lt
                xa = xg[:, lt, :]
                nc.tensor.matmul(
                    G0[:, :], xa[:, 0:P], xa,
                    start=(k == 0), stop=(k == NT - 1),
                )
                nc.tensor.matmul(
                    G1[:, :], xa[:, P:C], xa,
                    start=(k == 0), stop=(k == NT - 1),
                )
            # transposes of this chunk into X^T
            for cb in range(2):
                for h in range(LT // 4):
                    tg = pp_xt.tile([P, 4 * P], F32R, tag="xtT")
                    for i in range(4):
                        lt = h * 4 + i
                        nc.tensor.transpose(
                            tg[:, i * P:(i + 1) * P],
                            xg[:, lt, cb * P:(cb + 1) * P],
                            identr,
                        )
                    t0 = (g * LT + h * 4) * P
                    dst = st.XT[cb][:, t0:t0 + 4 * P]
                    if (g * 2 + cb + h) % 4 != 3:
                        nc.scalar.copy(dst, tg)
                    else:
                        nc.vector.tensor_copy(dst, tg)

        # copy G out of PSUM immediately so the next batch's Gram can start
        Gs0 = small.tile([P, C], F32, tag="gs0")
        Gs1 = small.tile([P, P], F32, tag="gs1")
        nc.scalar.copy(Gs0, G0)
        nc.scalar.copy(Gs1, G1[:, P:C])

        # column norms from the Gram diagonal
        scr = small.tile([P, P], F32, tag="scr")
        Q0 = small.tile([P, 1], F32, tag="q0")
        Q1 = small.tile([P, 1], F32, tag="q1")
        nc.vector.tensor_mul(scr, Gs0[:, 0:P], ident)
        nc.vector.tensor_reduce(Q0, scr, axis=X_AX, op=ALU.add)
        scr2 = small.tile([P, P], F32, tag="scr2")
        nc.vector.tensor_mul(scr2, Gs1, ident)
        nc.vector.tensor_reduce(Q1, scr2, axis=X_AX, op=ALU.add)
        sq0 = small.tile([P, 1], F32, tag="sq0")
        sq1 = small.tile([P, 1], F32, tag="sq1")
        nc.scalar.activation(sq0, Q0, AF.Ln, bias=cbias(EPS), scale=1.0)
        nc.scalar.activation(sq1, Q1, AF.Ln, bias=cbias(EPS), scale=1.0)
        s0 = small.tile([P, 1], F32, tag="s0")
        s1 = small.tile([P, 1], F32, tag="s1")
        nc.scalar.activation(s0, sq0, AF.Exp, bias=cbias(0.0), scale=-0.5)
        nc.scalar.activation(s1, sq1, AF.Exp, bias=cbias(0.0), scale=-0.5)

        # cor blocks: row-scale, transpose, scale again on the copy out
        u00 = small.tile([P, P], F32, tag="u00")
        u01 = small.tile([P, P], F32, tag="u01")
        u11 = small.tile([P, P], F32, tag="u11")
        nc.vector.tensor_scalar_mul(u00, Gs0[:, 0:P], s0)
        nc.vector.tensor_scalar_mul(u01, Gs0[:, P:C], s0)
        nc.vector.tensor_scalar_mul(u11, Gs1, s1)
        tgc = pp_mlp.tile([P, 3 * P], F32, tag="mlpT")
        nc.tensor.transpose(tgc[:, 0:P], u00, ident)
        nc.tensor.transpose(tgc[:, P:2 * P], u01, ident)
        nc.tensor.transpose(tgc[:, 2 * P:3 * P], u11, ident)
        m_in = mlp.tile([P, 3 * P], F32, tag="m_in", bufs=3)
        nc.vector.tensor_scalar_mul(m_in[:, 0:P], tgc[:, 0:P], s0)
        nc.vector.tensor_scalar_mul(m_in[:, P:2 * P], tgc[:, P:2 * P], s1)
        nc.vector.tensor_scalar_mul(m_in[:, 2 * P:3 * P], tgc[:, 2 * P:3 * P], s1)
        st.m_in = m_in

    # ---------------- stage B: MLP + normalize -> A'' ----------------
    def stage_b(b, st):
        m_in = st.m_in
        W = 3 * P

        def elu_pair(src, scale, bias):
            E = mlp.tile([P, W], F32, tag="e", bufs=6)
            R = mlp.tile([P, W], F32, tag="r", bufs=6)
            nc.scalar.activation(E, src, AF.Exp, bias=bias, scale=scale)
            nc.scalar.activation(R, src, AF.Relu, bias=bias, scale=scale)
            h = mlp.tile([P, W], F32, tag="h")
            nc.vector.scalar_tensor_tensor(
                h, in0=E, scalar=-1.0, in1=R, op0=ALU.add, op1=ALU.min
            )
            return h

        H1 = []
        for i in range(4):
            E = mlp.tile([P, W], F16, tag="e", bufs=6)
            R = mlp.tile([P, W], F16, tag="r", bufs=6)
            nc.scalar.activation(E, m_in, AF.Exp, bias=cbias(b1[i]), scale=w1[i])
            import os as _os
            if b1[i] == 0.0 and _os.environ.get("K_R1", "act") == "dve":
                nc.vector.tensor_scalar(
                    out=R, in0=m_in, scalar1=w1[i], scalar2=0.0,
                    op0=ALU.mult, op1=ALU.max,
                )
            else:
                nc.scalar.activation(R, m_in, AF.Relu, bias=cbias(b1[i]), scale=w1[i])
            h = mlp.tile([P, W], F16, tag=f"h1_{i}", bufs=1)
            nc.vector.scalar_tensor_tensor(
                h, in0=E, scalar=-1.0, in1=R, op0=ALU.add, op1=ALU.min
            )
            H1.append(h)
            if i % 2 == 1:
                yield

        H2 = []
        for j in range(4):
            import os as _os
            _l2g = _os.environ.get("K_L2G", "dve")
            if _l2g == "odd":
                eng = nc.gpsimd if j % 2 == 1 else nc.vector
            elif _l2g == "j3":
                eng = nc.gpsimd if j == 3 else nc.vector
            else:
                eng = nc.vector
            cur = mlp.tile([P, W], F16, tag="u", bufs=6)
            eng.tensor_scalar(
                out=cur, in0=H1[0], scalar1=W2[0][j], scalar2=b2[j],
                op0=ALU.mult, op1=ALU.add,
            )
            for i in range(1, 4):
                nxt = mlp.tile([P, W], F16, tag="u", bufs=6)
                eng.scalar_tensor_tensor(
                    nxt, in0=H1[i], scalar=W2[i][j], in1=cur,
                    op0=ALU.mult, op1=ALU.add,
                )
                cur = nxt
            E = mlp.tile([P, W], F16, tag="e", bufs=6)
            R = mlp.tile([P, W], F16, tag="r", bufs=6)
            nc.scalar.activation(E, cur, AF.Exp, bias=cbias(0.0))
            nc.scalar.activation(R, cur, AF.Relu, bias=cbias(0.0))
            h = mlp.tile([P, W], F16, tag=f"h2_{j}", bufs=1)
            nc.vector.scalar_tensor_tensor(
                h, in0=E, scalar=-1.0, in1=R, op0=ALU.add, op1=ALU.min
            )
            H2.append(h)
            yield

        import os as _os
        _l3 = nc.gpsimd if _os.environ.get("K_L3", "dve") == "gps" else nc.vector
        cur = mlp.tile([P, W], F16, tag="u", bufs=6)
        _l3.tensor_scalar(
            out=cur, in0=H2[0], scalar1=wp2[0], scalar2=bp2,
            op0=ALU.mult, op1=ALU.add,
        )
        for j in range(1, 4):
            nxt = mlp.tile([P, W], F16, tag="u", bufs=6)
            _l3.scalar_tensor_tensor(
                nxt, in0=H2[j], scalar=wp2[j], in1=cur, op0=ALU.mult, op1=ALU.add
            )
            cur = nxt
        eneg = mlp.tile([P, W], F16, tag="e", bufs=6)
        nc.scalar.activation(eneg, cur, AF.Exp, bias=cbias(0.0), scale=-1.0)
        wden = mlp.tile([P, W], F32, tag="u", bufs=6)
        nc.vector.tensor_scalar_add(wden, eneg, 1.0)
        a_out = mlp.tile([P, W], F32, tag="a_out")
        nc.vector.reciprocal(a_out, wden)
        yield

        # A01 = A10^T  (att pre-normalization is symmetric)
        t01p = pp_mlp.tile([P, 3 * P], F32, tag="mlpT")
        nc.tensor.transpose(t01p[:, 0:P], a_out[:, P:2 * P], ident)
        t01 = small.tile([P, P], F32, tag="t01")
        S01 = small.tile([P, 1], F32, tag="s01")
        nc.scalar.copy(t01, t01p[:, 0:P])
        nc.vector.tensor_reduce(S01, t01, axis=X_AX, op=ALU.add)

        # column sums via row sums (symmetry)
        Sa = small.tile([P, 1], F32, tag="sa")
        nc.vector.tensor_reduce(Sa, a_out[:, 0:P], axis=X_AX, op=ALU.add)
        Sc0 = small.tile([P, 1], F32, tag="sc0")
        nc.vector.tensor_add(Sc0, Sa, S01)
        Sc1 = small.tile([P, 1], F32, tag="sc1")
        nc.vector.tensor_reduce(Sc1, a_out[:, P:3 * P], axis=X_AX, op=ALU.add)
        yield
        R0 = small.tile([P, 1], F32, tag="r0")
        R1 = small.tile([P, 1], F32, tag="r1")
        t0 = small.tile([P, 1], F32, tag="t0")
        t1 = small.tile([P, 1], F32, tag="t1")
        nc.vector.tensor_scalar_add(t0, Sc0, EPS)
        nc.vector.tensor_scalar_add(t1, Sc1, EPS)
        nc.vector.reciprocal(R0, t0)
        nc.vector.reciprocal(R1, t1)

        # U = R * (A * offdiag-mask), blockwise
        n00 = small.tile([P, P], F32, tag="n00")
        n11 = small.tile([P, P], F32, tag="n11")
        nc.vector.tensor_mul(n00, a_out[:, 0:P], od)
        nc.vector.tensor_mul(n11, a_out[:, 2 * P:3 * P], od)
        v00 = small.tile([P, P], F32, tag="v00")
        v01 = small.tile([P, P], F32, tag="v01")
        v10 = small.tile([P, P], F32, tag="v10")
        v11 = small.tile([P, P], F32, tag="v11")
        nc.vector.tensor_scalar_mul(v00, n00, R0)
        nc.vector.tensor_scalar_mul(v01, t01, R0)
        nc.vector.tensor_scalar_mul(v10, a_out[:, P:2 * P], R1)
        nc.vector.tensor_scalar_mul(v11, n11, R1)

        # A'' = U^T + I
        p0 = pp_mlp.tile([P, 3 * P], F32, tag="mlpT")
        nc.tensor.transpose(p0[:, 0:P], v00, ident)
        nc.tensor.transpose(p0[:, P:2 * P], v10, ident)
        p1 = pp_mlp.tile([P, 3 * P], F32, tag="mlpT")
        nc.tensor.transpose(p1[:, 0:P], v01, ident)
        nc.tensor.transpose(p1[:, P:2 * P], v11, ident)
        A0 = apool.tile([P, C], F32R, tag="A0")
        A1 = apool.tile([P, C], F32R, tag="A1")
        nc.vector.scalar_tensor_tensor(
            A0[:, 0:P], in0=p0[:, 0:P], scalar=1.0, in1=ident,
            op0=ALU.mult, op1=ALU.add,
        )
        nc.scalar.copy(A0[:, P:C], p0[:, P:2 * P])
        nc.scalar.copy(A1[:, 0:P], p1[:, 0:P])
        nc.vector.scalar_tensor_tensor(
            A1[:, P:C], in0=p1[:, P:2 * P], scalar=1.0, in1=ident,
            op0=ALU.mult, op1=ALU.add,
        )
        st.A = (A0, A1)

    # ---------------- stage C: out = X @ A'' ----------------
    def stage_c(b, st):
        A0, A1 = st.A
        XT0, XT1 = st.XT
        for g in range(NCHUNK):
            ost = ostage.tile([P, LT, C], F32, tag="ost")
            for lt in range(LT):
                yield
                tb = g * LT + lt
                po = pp_out.tile([P, C], F32, tag="po")
                nc.tensor.matmul(
                    po, XT0[:, tb * P:(tb + 1) * P], A0,
                    start=True, stop=False,
                )
                nc.tensor.matmul(
                    po, XT1[:, tb * P:(tb + 1) * P], A1,
                    start=False, stop=True,
                )
                if lt % 2 == 0:
                    nc.scalar.copy(ost[:, lt, :], po)
                else:
                    nc.vector.tensor_copy(ost[:, lt, :], po)
            dst = o_d[b, g * LT * P:(g + 1) * LT * P, :].rearrange(
                "(lt p) c -> p lt c", p=P
            )
            nc.sync.dma_start(out=dst, in_=ost)

    # ---------------- software-pipelined emission ----------------
    import os as _os
    nb = int(_os.environ.get("KERNEL_NBATCH", str(B_PER_CORE)))
    _ = _os
    sts = [_St() for _ in range(B_PER_CORE)]
    del STAGE_RANGES[:]
    _ga, _gb, _gc = stage_a, stage_b, stage_c

    def drive(*gens):
        live = list(gens)
        while live:
            for g in list(live):
                try:
                    next(g)
                except StopIteration:
                    live.remove(g)

    def _tag(label, gen):
        _mark(nc, label)
        return gen

    def stage_a(b, st):
        _mark(nc, f"A{b}")
        drive(_ga(b, st))

    def stage_b(b, st):
        _mark(nc, f"B{b}")
        drive(_gb(b, st))

    def stage_c(b, st):
        _mark(nc, f"C{b}")
        drive(_gc(b, st))

    if nb == 1:
        stage_a(0, sts[0]); stage_b(0, sts[0]); stage_c(0, sts[0])
    elif nb == 2:
        stage_a(0, sts[0]); stage_a(1, sts[1])
        stage_b(0, sts[0]); stage_c(0, sts[0])
        stage_b(1, sts[1]); stage_c(1, sts[1])
    elif _os.environ.get("K_PIPE", "i") == "i":
        # op-granular interleave: fill MLP-chain stalls with bulk work
        _mark(nc, "A0")
        drive(_ga(0, sts[0]))
        _mark(nc, "A1B0")
        drive(_gb(0, sts[0]), _ga(1, sts[1]))
        _mark(nc, "B1C0A2")
        drive(_gb(1, sts[1]), _gc(0, sts[0]), _ga(2, sts[2]))
        _mark(nc, "B2C1A3")
        drive(_gb(2, sts[2]), _gc(1, sts[1]), _ga(3, sts[3]))
        _mark(nc, "B3C2")
        drive(_gb(3, sts[3]), _gc(2, sts[2]))
        _mark(nc, "C3")
        drive(_gc(3, sts[3]))
    elif _os.environ.get("K_PIPE", "i") == "3":
        stage_a(0, sts[0])
        stage_a(1, sts[1])
        stage_b(0, sts[0])
        stage_a(2, sts[2])
        stage_c(0, sts[0])
        stage_b(1, sts[1])
        stage_a(3, sts[3])
        stage_c(1, sts[1])
        stage_b(2, sts[2])
        stage_c(2, sts[2])
        stage_b(3, sts[3])
        stage_c(3, sts[3])
    else:
        stage_a(0, sts[0])
        stage_a(1, sts[1])
        stage_b(0, sts[0])
        stage_c(0, sts[0])
        stage_a(2, sts[2])
        stage_b(1, sts[1])
        stage_c(1, sts[1])
        stage_a(3, sts[3])
        stage_b(2, sts[2])
        stage_c(2, sts[2])
        stage_b(3, sts[3])
        stage_c(3, sts[3])

    ctx.close()


_ACT_TABLES_PATCHED = False


def _pin_act_table():
    """Force the act-table chooser onto natural_log_exp_and_others (which
    covers Copy/Ln/Exp/Relu) so the whole kernel needs ONE table load
    instead of thrashing between exp_and_others and natural_log."""
    global _ACT_TABLES_PATCHED
    if _ACT_TABLES_PATCHED:
        return
    from concourse import hw_specs
    import concourse.bacc as bacc_mod
    orig = hw_specs.get_activation_tables
    mine = {AF.Copy, AF.Ln, AF.Exp, AF.Relu, AF.Identity}
    keep = "natural_log_exp_and_others"

    def patched(arch):
        tabs = orig(arch)
        if keep not in tabs or not mine <= tabs[keep]:
            return tabs
        return {
            name: (s if name == keep else s - mine)
            for name, s in tabs.items()
        }

    bacc_mod.get_activation_tables = patched
    _ACT_TABLES_PATCHED = True


def build_program(inputs):
    """Build + compile the SPMD Bass program with folded params baked in."""
    _pin_act_table()
    w1, b1, W2, b2, wp2, bp2 = _fold_params(inputs)
    nc = bacc.Bacc(
        "TRN2",
        target_bir_lowering=False,
        debug=False,
        enable_asserts=False,
        num_devices=N_CORES,
    )
    x_d = nc.dram_tensor("x", (B_PER_CORE, T, C), F32, kind="ExternalInput").ap()
    o_d = nc.dram_tensor("out", (B_PER_CORE, T, C), F32, kind="ExternalOutput").ap()
    with tile.TileContext(nc) as tc:
        _build_kernel(tc, o_d, x_d, w1, b1, W2, b2, wp2, bp2)
    nc.compile()
    return nc


def run_device(nc, x, trace=False, **kw):
    """Run the compiled program over the 8 cores; return (out, results)."""
    x = np.asarray(x, np.float32)
    in_maps = [
        {"x": np.ascontiguousarray(x[c * B_PER_CORE:(c + 1) * B_PER_CORE])}
        for c in range(N_CORES)
    ]
    res = bass_utils.run_bass_kernel_spmd(
        nc, in_maps, core_ids=list(range(N_CORES)), trace=trace, **kw
    )
    out = np.concatenate(
        [res.results[c]["out"] for c in range(N_CORES)], axis=0
    )
    return out, res


def host_penalty(inputs):
    """Monotonicity penalty on 21 keypoints; float32 math mirroring reference."""
    f32 = np.float32
    W1 = np.asarray(inputs["W1"], f32)
    b1 = np.asarray(inputs["b1"], f32)
    W2 = np.asarray(inputs["W2"], f32)
    b2 = np.asarray(inputs["b2"], f32)
    gamma = np.asarray(inputs["gamma"], f32)
    beta = np.asarray(inputs["beta"], f32)
    mmean = np.asarray(inputs["mmean"], f32)
    mvar = np.asarray(inputs["mvar"], f32)
    Wp = np.asarray(inputs["Wp"], f32)
    bp = np.asarray(inputs["bp"], f32)

    def elu(v):
        return np.where(v > 0, v, np.expm1(v)).astype(f32)

    z = np.linspace(-1.0, 1.0, 21).astype(f32).reshape(-1, 1)
    h = elu(z @ W1 + b1)
    h = elu(h @ W2 + b2)
    h = (h - mmean) * (f32(1.0) / np.sqrt(mvar + f32(BN_EPS))) * gamma + beta
    o = h @ Wp + bp
    kout = (f32(1.0) / (f32(1.0) + np.exp(-o)))[:, 0]
    dL = kout[1:11] - kout[:10]
    dR = kout[11:] - kout[10:-1]
    pen = f32(0.5) * f32(PENALTY_RATE) * np.mean(
        np.abs(dL) - dL + np.abs(dR) - dR, dtype=f32
    )
    return f32(pen)


def kernel(**inputs):
    x = np.asarray(inputs["x"], np.float32)
    nc = build_program(inputs)
    out, _ = run_device(nc, x)
    penalty = host_penalty(inputs)
    return out, penalty
